# revision 3
# baseline (speedup 1.0000x reference)
"""ClusterGCN 2-layer kernel for 8 Trainium2 NeuronCores.

Strategy:
 - Exploit linearity: project x (770ch) down to 32ch FIRST (z1 = x @ W_out1.T),
   then message-pass on 32-dim vectors (24x less gather traffic).
 - Edge weight = deg_inv[dest] (uniform per destination) => aggregate raw
   neighbor sums, scale once per destination.
 - Nodes degree-sorted and dealt across 8 cores; per-destination padded slot
   lists (gather indices) shared by both layers.
 - Device: z1|r1 via PE matmul, AllGather z1 (row-major table, 256B stride),
   one dma_gather per chunk of destination tiles (int16 signed wrapped
   indices with +32768-row base), strided middle-axis tensor_reduce,
   elementwise assembly of h, AllGather h, same gathers again, PE transpose,
   final f32r matmul to [128, 770] output tiles.
"""

import os
import sys
import types

import numpy as np

# ---------------------------------------------------------------------------
# environment shims (axon NTFF hook + no artifact bucket)
# ---------------------------------------------------------------------------
if "antenv.axon_hooks" not in sys.modules:
    _mod = types.ModuleType("antenv.axon_hooks")
    _hook_store = [None]
    _mod.set_axon_ntff_profile_hook = lambda h: _hook_store.__setitem__(0, h)
    _mod.get_axon_ntff_profile_hook = lambda: _hook_store[0]
    try:
        import antenv

        antenv.axon_hooks = _mod
        sys.modules["antenv.axon_hooks"] = _mod
        from trn_agent_boot.trn_boot import _ntff_profile_via_ctypes

        _mod.set_axon_ntff_profile_hook(
            _ntff_profile_via_ctypes("/opt/axon/libaxon_pjrt.so")
        )
    except Exception:
        pass

import concourse.bacc as bacc
import concourse.bass as bass
import concourse.bass_utils as bass_utils
import concourse.mybir as mybir
import concourse.tile as tile
from concourse.bass_utils import run_bass_kernel_spmd
from concourse.masks import make_identity

bass_utils.upload_artifacts = lambda tmpdir: tmpdir

# ---------------------------------------------------------------------------
# problem constants (hardcoded per the harness contract)
# ---------------------------------------------------------------------------
N = 50000
E = 400000
IN_CH = 770
HID = 32
DIAG_LAMBDA = 1.0
NCORES = 8
P = 128
T = 49  # destination tiles per core
LOCAL = T * P  # 6272 rows per core block
NRANK = T * 1024  # 50176 ranks total
NROWS = NCORES * LOCAL  # 50176 gather-table rows
BASE = 32768  # gather base-row offset (signed int16 wrap)
ROWE = 64  # table row stride in f32 elems (256B)
XCH = 112  # input-channel chunk (7 chunks x 112 = 784)
NCHUNK_IN = 7
XROWS = NCHUNK_IN * XCH  # 784 = 770 + ones row + 13 zero rows
MAXCOLS = 40  # max gather columns per dma_gather chunk

F32 = mybir.dt.float32
F32R = mybir.dt.float32r
I16 = mybir.dt.int16


def _host_prep(x, edge_index):
    """Degree stats, node permutation, gather slot tables, xTe."""
    row = np.asarray(edge_index[0], dtype=np.int64)
    col = np.asarray(edge_index[1], dtype=np.int64)
    ns = row != col
    r_, c_ = row[ns], col[ns]
    indeg = np.bincount(c_, minlength=N)
    deg = (indeg + 1).astype(np.float64)
    dinv = (1.0 / deg).astype(np.float32)

    order = np.argsort(-indeg, kind="stable")  # rank -> node
    rank_of = np.empty(N, dtype=np.int64)
    rank_of[order] = np.arange(N)

    r_all = np.arange(NRANK)
    core_of_rank = (r_all % 1024) // 128
    l_of_rank = (r_all // 1024) * 128 + (r_all % 128)
    g_of_rank = core_of_rank * LOCAL + l_of_rank
    g_of_node = g_of_rank[rank_of]  # node -> table row

    indeg_rank = np.zeros(NRANK, dtype=np.int64)
    indeg_rank[:N] = indeg[order]
    K = np.maximum(indeg_rank[np.arange(T) * 1024], 1).astype(np.int64)
    off = np.concatenate([[0], np.cumsum(K)])
    totk = int(off[-1])

    # chunks of consecutive tiles, each <= MAXCOLS gather columns (+1 pad col)
    chunks = []
    t0 = 0
    acc = 0
    for t in range(T):
        if acc and acc + K[t] > MAXCOLS - 1:
            chunks.append((t0, t))
            t0 = t
            acc = 0
        acc += int(K[t])
    chunks.append((t0, T))

    # slot table [8, totk, 128] of table-row g values, init to pad rows
    padg = g_of_rank[N:NRANK]  # 176 all-zero rows (cores 6,7 tails)
    init = padg[np.arange(8 * totk * 128) % len(padg)]
    slot_g = init.reshape(8, totk, 128)

    dest_rank = rank_of[c_]
    sidx = np.argsort(dest_rank, kind="stable")
    dr = dest_rank[sidx]
    src_g = g_of_node[r_[sidx]]
    cnt = np.bincount(dr, minlength=NRANK)
    cum = np.concatenate([[0], np.cumsum(cnt)])
    within = np.arange(len(dr)) - cum[dr]
    t_d = dr // 1024
    c_d = (dr % 1024) // 128
    p_d = dr % 128
    colg = off[t_d] + within
    slot_g[c_d, colg, p_d] = src_g

    # final per-core index arrays with chunk pad columns appended
    wrapped = (slot_g - BASE).astype(np.int16)
    padcol = (padg[np.arange(128) % len(padg)] - BASE).astype(np.int16)  # >0
    per_core_idxs = []
    chunk_meta = []  # (idx_col_off, cols_ch, runs, gbuf_tile_offs)
    for c in range(NCORES):
        parts = []
        icol = 0
        for (a, b) in chunks:
            cols_ch = int(off[b] - off[a]) + 1
            parts.append(wrapped[c, off[a] : off[b], :])
            parts.append(padcol[None, :])
            if c == 0:
                # reduce runs: consecutive tiles with equal K
                runs = []
                t = a
                while t < b:
                    t2 = t
                    while t2 < b and K[t2] == K[t]:
                        t2 += 1
                    runs.append(
                        (int(off[t] - off[a]), t2 - t, int(K[t]), t)
                    )  # (col_off_in_chunk, ntiles, K, tile0)
                    t = t2
                chunk_meta.append((icol, cols_ch, runs))
            icol += cols_ch
        allcols = np.concatenate(parts, axis=0)  # [TOTC, 128]
        totc = allcols.shape[0]
        flat = allcols.reshape(-1)  # position j = colc*128 + p
        a16 = np.zeros((16, totc * 8), dtype=np.int16)
        j = np.arange(totc * 128)
        a16[j % 16, j // 16] = flat
        per_core_idxs.append(np.tile(a16, (8, 1)))
    totc_all = per_core_idxs[0].shape[1] // 8

    # per-core dinv [128, T]
    dinv_rank = np.zeros(NRANK, dtype=np.float32)
    dinv_rank[:N] = dinv[order]
    dpc = np.zeros((NCORES, P, T), dtype=np.float32)
    for c in range(NCORES):
        rr = (np.arange(T) * 1024)[None, :] + c * 128 + np.arange(P)[:, None]
        dpc[c] = dinv_rank[rr]

    # xTe [XROWS, NROWS]: col g holds x[node].T; row 770 = 1 for real cols
    xTe = np.zeros((XROWS, NROWS), dtype=np.float32)
    xTe[:IN_CH, g_of_node] = np.asarray(x, dtype=np.float32).T
    xTe[IN_CH, g_of_node] = 1.0

    layout = {
        "K": K,
        "chunks": chunks,
        "chunk_meta": chunk_meta,
        "totc": totc_all,
        "off": off,
    }
    return layout, per_core_idxs, dpc, xTe, g_of_node


def dma_gather_raw(nc, out_ap, in_ap, idxs_ap, num_idxs, elem_size, elem_step, queue_num=0):
    """bass dma_gather without the %256 elem-size assert (non-transpose, HBM
    source, multi-packet). Row stride (elem_step * 4B) must be %256 == 0."""
    gp = nc.gpsimd
    stride_bytes = elem_step * mybir.dt.size(in_ap.dtype)
    assert stride_bytes % 256 == 0 and stride_bytes // 256 < 256
    return gp.add_instruction(
        mybir.InstDMAGatherAnt(
            name=nc.get_next_instruction_name(),
            ins=[
                *gp.lower_ap_dma(in_ap, for_custom_bir_dma=True),
                gp.lower_ap(idxs_ap),
                gp.lower_val_access(gp.to_reg(num_idxs)),
            ],
            outs=[gp.lower_ap(out_ap)],
            transpose=False,
            num_idxs=num_idxs,
            elem_size=elem_size,
            stride_bytes_256=stride_bytes // 256,
            gen_mode=0,
            single_packet=False,
            queue_num=queue_num,
            sbuf_tokens_per_rank=0,
            sbuf_free_dim_per_rank=0,
            sbuf_free_dim_pad_per_rank=0,
            sbuf_byte_offset=0,
        )
    )


def build(layout):
    K = layout["K"]
    chunks = layout["chunks"]
    chunk_meta = layout["chunk_meta"]
    totc = layout["totc"]
    off = layout["off"]

    nc = bacc.Bacc("TRN2", num_devices=NCORES, debug=False, num_swdge_queues=4)

    xTe = nc.dram_tensor("xTe", [XROWS, LOCAL], F32R, kind="ExternalInput")
    w1 = nc.dram_tensor("w1", [XCH, NCHUNK_IN * 64], F32R, kind="ExternalInput")
    w2 = nc.dram_tensor("w2", [64, IN_CH], F32R, kind="ExternalInput")
    idxs = nc.dram_tensor("idxs", [P, totc * 8], I16, kind="ExternalInput")
    dinv_in = nc.dram_tensor("dinv", [P, T], F32, kind="ExternalInput")
    out = nc.dram_tensor("out", [LOCAL, IN_CH], F32, kind="ExternalOutput")

    z1loc = nc.dram_tensor("z1loc", [LOCAL, ROWE], F32)
    hloc = nc.dram_tensor("hloc", [LOCAL, ROWE], F32)
    z1g = nc.dram_tensor("z1g", [NROWS, ROWE], F32, addr_space="Shared")
    hg = nc.dram_tensor("hg", [NROWS, ROWE], F32, addr_space="Shared")

    stsizes = [512] * 12 + [128]  # node supertiles (6272 total)

    with tile.TileContext(nc) as tc:
        with (
            tc.tile_pool(name="persist", bufs=1) as pp,
            tc.tile_pool(name="xload", bufs=3) as xp,
            tc.tile_pool(name="gather", bufs=4) as gp_pool,
            tc.tile_pool(name="work", bufs=2) as wp,
            tc.tile_pool(name="outsb", bufs=3) as op_pool,
            tc.tile_pool(name="l1ps", bufs=2, space="PSUM") as l1ps,
            tc.tile_pool(name="trps", bufs=2, space="PSUM") as trps,
            tc.tile_pool(name="outps", bufs=2, space="PSUM") as outps,
        ):
            # ---- persistent loads ----
            w1_sb = pp.tile([XCH, NCHUNK_IN * 64], F32R)
            nc.sync.dma_start(w1_sb[:], w1[:])
            w2_sb = pp.tile([64, IN_CH], F32R)
            nc.sync.dma_start(w2_sb[:], w2[:])
            idxs_sb = pp.tile([P, totc * 8], I16)
            nc.sync.dma_start(idxs_sb[:], idxs[:])
            dinv_sb = pp.tile([P, T], F32)
            nc.sync.dma_start(dinv_sb[:], dinv_in[:])
            ident = pp.tile([P, P], F32)
            make_identity(nc, ident)

            z1r_sb = pp.tile([P, T * 64], F32)  # [z1 | r1+b1] per tile
            slotred = pp.tile([P, T * HID], F32)
            slotred2 = pp.tile([P, T * HID], F32)
            h_sb = pp.tile([P, T * HID], F32)
            tmp_sb = pp.tile([P, T * HID], F32)
            ag2h = pp.tile([P, T * 64], F32)  # [agg2 | h] per tile

            w1v = w1_sb[:].rearrange("p (k c) -> p k c", k=NCHUNK_IN)

            # ---- layer-1 matmul: z1|r1b = xTe_aug @ W1cat ----
            tglob = 0
            for st, stn in enumerate(stsizes):
                xsb = xp.tile([XCH, NCHUNK_IN, 512], F32R, tag="xsb")
                src = xTe.ap().rearrange("(k q) n -> q k n", q=XCH)[
                    :, :, st * 512 : st * 512 + stn
                ]
                nc.sync.dma_start(xsb[:, :, :stn], src)
                for tloc in range(stn // 128):
                    ps = l1ps.tile([P, 64], F32, space="PSUM")
                    for k in range(NCHUNK_IN):
                        nc.tensor.matmul(
                            out=ps[:],
                            lhsT=xsb[:, k, tloc * 128 : (tloc + 1) * 128],
                            rhs=w1v[:, k, :],
                            start=(k == 0),
                            stop=(k == NCHUNK_IN - 1),
                        )
                    nc.vector.tensor_copy(
                        z1r_sb[:, tglob * 64 : (tglob + 1) * 64], ps[:]
                    )
                    tglob += 1

            # ---- store z1 rows, AllGather ----
            z1v = z1r_sb[:].rearrange("p (t d) -> p t d", t=T)
            z1dst = z1loc.ap().rearrange("(t p) c -> p t c", p=P)[:, :, 0:HID]
            nc.sync.dma_start(z1dst, z1v[:, :, 0:HID])
            nc.gpsimd.collective_compute(
                "AllGather",
                mybir.AluOpType.bypass,
                replica_groups=[list(range(NCORES))],
                ins=[z1loc.ap().opt()],
                outs=[z1g.ap().opt()],
            )

            # ---- gather + reduce helper ----
            def gather_layer(table, dest_red, sems):
                for ci, (icol, cols_ch, runs) in enumerate(chunk_meta):
                    gbuf = gp_pool.tile([P, MAXCOLS, HID], F32, tag="gbuf")
                    sem = sems[ci]
                    with tc.tile_critical():
                        dma_gather_raw(
                            nc,
                            gbuf[:, :cols_ch, :],
                            table[BASE:, :],
                            idxs_sb[:, icol * 8 : (icol + cols_ch) * 8],
                            num_idxs=cols_ch * 128,
                            elem_size=HID,
                            elem_step=ROWE,
                            queue_num=ci % 4,
                        ).then_inc(sem, 16)
                    with tc.tile_critical():
                        nc.vector.wait_ge(sem, 16)
                        for (coff, nt, kk, t0) in runs:
                            inv = gbuf[:, coff : coff + nt * kk, :].rearrange(
                                "p (t k) c -> p t c k", k=kk
                            )
                            nc.vector.tensor_reduce(
                                out=dest_red[:, t0 * HID : (t0 + nt) * HID],
                                in_=inv,
                                axis=mybir.AxisListType.X,
                                op=mybir.AluOpType.add,
                            )

            sems1 = [nc.alloc_semaphore(f"g1_{i}") for i in range(len(chunk_meta))]
            gather_layer(z1g, slotred, sems1)

            # ---- h = relu(dinv*(slotred + 2*z1) + r1b) ----
            dinv_b = dinv_sb[:].to_broadcast([P, T, HID])
            sr_v = slotred[:].rearrange("p (t c) -> p t c", t=T)
            tmp_v = tmp_sb[:].rearrange("p (t c) -> p t c", t=T)
            h_v = h_sb[:].rearrange("p (t c) -> p t c", t=T)
            nc.vector.tensor_scalar(
                out=tmp_v, in0=z1v[:, :, 0:HID], scalar1=2.0, scalar2=None,
                op0=mybir.AluOpType.mult,
            )
            nc.vector.tensor_tensor(
                out=tmp_sb[:], in0=tmp_sb[:], in1=slotred[:], op=mybir.AluOpType.add
            )
            nc.vector.tensor_tensor(
                out=tmp_v, in0=tmp_v, in1=dinv_b, op=mybir.AluOpType.mult
            )
            nc.vector.tensor_tensor(
                out=tmp_v, in0=tmp_v, in1=z1v[:, :, HID:64],
                op=mybir.AluOpType.add,
            )
            nc.vector.tensor_scalar(
                out=h_sb[:], in0=tmp_sb[:], scalar1=0.0, scalar2=None,
                op0=mybir.AluOpType.max,
            )

            ag2h_v = ag2h[:].rearrange("p (t d) -> p t d", t=T)
            nc.vector.tensor_copy(ag2h_v[:, :, HID:64], h_v)

            # ---- store h rows, AllGather ----
            hdst = hloc.ap().rearrange("(t p) c -> p t c", p=P)[:, :, 0:HID]
            nc.sync.dma_start(hdst, h_v)
            nc.gpsimd.collective_compute(
                "AllGather",
                mybir.AluOpType.bypass,
                replica_groups=[list(range(NCORES))],
                ins=[hloc.ap().opt()],
                outs=[hg.ap().opt()],
            )

            sems2 = [nc.alloc_semaphore(f"g2_{i}") for i in range(len(chunk_meta))]
            gather_layer(hg, slotred2, sems2)

            # ---- agg2 = dinv*(slotred2 + 2*h) -> ag2h[:, :, 0:HID] ----
            sr2_v = slotred2[:].rearrange("p (t c) -> p t c", t=T)
            nc.vector.tensor_scalar(
                out=tmp_sb[:], in0=h_sb[:], scalar1=2.0, scalar2=None,
                op0=mybir.AluOpType.mult,
            )
            nc.vector.tensor_tensor(
                out=tmp_sb[:], in0=tmp_sb[:], in1=slotred2[:], op=mybir.AluOpType.add
            )
            nc.vector.tensor_tensor(
                out=ag2h_v[:, :, 0:HID], in0=tmp_v, in1=dinv_b,
                op=mybir.AluOpType.mult,
            )

            # ---- per tile: transpose -> catT, matmul, copy out, DMA ----
            for t in range(T):
                tp = trps.tile([64, P], F32, space="PSUM")
                nc.tensor.transpose(
                    out=tp[:], in_=ag2h[:, t * 64 : (t + 1) * 64], identity=ident[:]
                )
                catT = wp.tile([64, P], F32R, tag="catT")
                nc.vector.tensor_copy(catT[:], tp[:])
                pso = outps.tile([P, IN_CH], F32, space="PSUM")
                nc.tensor.matmul(
                    out=pso[:, 0:512], lhsT=catT[:], rhs=w2_sb[:, 0:512],
                    start=True, stop=True,
                )
                nc.tensor.matmul(
                    out=pso[:, 512:IN_CH], lhsT=catT[:], rhs=w2_sb[:, 512:IN_CH],
                    start=True, stop=True,
                )
                osb = op_pool.tile([P, IN_CH], F32, tag="osb")
                if t % 2 == 0:
                    nc.vector.tensor_copy(osb[:], pso[:])
                else:
                    nc.scalar.copy(osb[:], pso[:])
                nc.sync.dma_start(out[t * P : (t + 1) * P, :], osb[:])

    nc.compile()
    return nc


_CACHE = {}


def kernel(x, edge_index, W_out1, b_out1, W_root1, W_out2, b_out2, W_root2):
    x = np.asarray(x, dtype=np.float32)
    layout, per_core_idxs, dpc, xTe, g_of_node = _host_prep(x, edge_index)

    # weight tensors
    w1cat = np.zeros((XROWS, 64), dtype=np.float32)
    w1cat[:IN_CH, :HID] = np.asarray(W_out1, dtype=np.float32).T
    w1cat[:IN_CH, HID:] = np.asarray(W_root1, dtype=np.float32).T
    w1cat[IN_CH, HID:] = np.asarray(b_out1, dtype=np.float32)
    # rearrange to [XCH, NCHUNK_IN*64]: [p, k*64+c] = w1cat[k*XCH+p, c]
    w1_dev = np.ascontiguousarray(
        w1cat.reshape(NCHUNK_IN, XCH, 64).transpose(1, 0, 2).reshape(XCH, -1)
    )
    w2_dev = np.concatenate(
        [np.asarray(W_out2, dtype=np.float32).T, np.asarray(W_root2, dtype=np.float32).T],
        axis=0,
    )  # [64, 770]

    key = (tuple(int(k) for k in layout["K"]), tuple(layout["chunks"]))
    if key not in _CACHE:
        _CACHE[key] = build(layout)
    nc = _CACHE[key]

    in_maps = []
    for c in range(NCORES):
        in_maps.append(
            {
                "xTe": np.ascontiguousarray(xTe[:, c * LOCAL : (c + 1) * LOCAL]),
                "w1": w1_dev,
                "w2": np.ascontiguousarray(w2_dev),
                "idxs": per_core_idxs[c],
                "dinv": dpc[c],
            }
        )

    trace = os.environ.get("GCN_TRACE", "0") == "1"
    res = run_bass_kernel_spmd(
        nc, in_maps, core_ids=list(range(NCORES)), trace=trace
    )
    if trace and res.exec_time_ns is not None:
        print(f"HW exec time: {res.exec_time_ns} ns")
        kernel.last_exec_time_ns = res.exec_time_ns

    out = np.empty((N, IN_CH), dtype=np.float32)
    # table row g -> node
    node_of_g = np.full(NROWS, -1, dtype=np.int64)
    node_of_g[g_of_node] = np.arange(N)
    for c in range(NCORES):
        gs = np.arange(c * LOCAL, (c + 1) * LOCAL)
        nodes = node_of_g[gs]
        m = nodes >= 0
        out[nodes[m]] = res.results[c]["out"][m]
    out += np.asarray(b_out2, dtype=np.float32)[None, :]
    return out


# revision 4
# speedup vs baseline: 1.0761x; 1.0761x over previous
"""ClusterGCN 2-layer kernel for 8 Trainium2 NeuronCores.

Strategy:
 - Exploit linearity: project x (770ch) down to 32ch FIRST (z1 = x @ W_out1.T),
   then message-pass on 32-dim vectors (24x less gather traffic).
 - Edge weight = deg_inv[dest] (uniform per destination) => aggregate raw
   neighbor sums, scale once per destination.
 - Nodes degree-sorted and dealt across 8 cores; per-destination padded slot
   lists (gather indices) shared by both layers.
 - Device: z1|r1 via PE matmul, AllGather z1 (row-major table, 256B stride),
   one dma_gather per chunk of destination tiles (int16 signed wrapped
   indices with +32768-row base), strided middle-axis tensor_reduce,
   elementwise assembly of h, AllGather h, same gathers again, PE transpose,
   final f32r matmul to [128, 770] output tiles.
"""

import os
import sys
import types

import numpy as np

# ---------------------------------------------------------------------------
# environment shims (axon NTFF hook + no artifact bucket)
# ---------------------------------------------------------------------------
if "antenv.axon_hooks" not in sys.modules:
    _mod = types.ModuleType("antenv.axon_hooks")
    _hook_store = [None]
    _mod.set_axon_ntff_profile_hook = lambda h: _hook_store.__setitem__(0, h)
    _mod.get_axon_ntff_profile_hook = lambda: _hook_store[0]
    try:
        import antenv

        antenv.axon_hooks = _mod
        sys.modules["antenv.axon_hooks"] = _mod
        from trn_agent_boot.trn_boot import _ntff_profile_via_ctypes

        _mod.set_axon_ntff_profile_hook(
            _ntff_profile_via_ctypes("/opt/axon/libaxon_pjrt.so")
        )
    except Exception:
        pass

import concourse.bacc as bacc
import concourse.bass as bass
import concourse.bass_utils as bass_utils
import concourse.mybir as mybir
import concourse.tile as tile
from concourse.bass_utils import run_bass_kernel_spmd
from concourse.masks import make_identity

bass_utils.upload_artifacts = lambda tmpdir: tmpdir

# ---------------------------------------------------------------------------
# problem constants (hardcoded per the harness contract)
# ---------------------------------------------------------------------------
N = 50000
E = 400000
IN_CH = 770
HID = 32
DIAG_LAMBDA = 1.0
NCORES = 8
P = 128
T = 49  # destination tiles per core
LOCAL = T * P  # 6272 rows per core block
NRANK = T * 1024  # 50176 ranks total
NROWS = NCORES * LOCAL  # 50176 gather-table rows
BASE = 32768  # gather base-row offset (signed int16 wrap)
ROWE = 64  # table row stride in f32 elems (256B)
XCH = 112  # input-channel chunk (7 chunks x 112 = 784)
NCHUNK_IN = 7
XROWS = NCHUNK_IN * XCH  # 784 = 770 + ones row + 13 zero rows
MAXCOLS = 40  # max gather columns per dma_gather chunk

F32 = mybir.dt.float32
F32R = mybir.dt.float32r
I16 = mybir.dt.int16


def _host_prep(x, edge_index):
    """Degree stats, node permutation, gather slot tables, xTe."""
    row = np.asarray(edge_index[0], dtype=np.int64)
    col = np.asarray(edge_index[1], dtype=np.int64)
    ns = row != col
    r_, c_ = row[ns], col[ns]
    indeg = np.bincount(c_, minlength=N)
    deg = (indeg + 1).astype(np.float64)
    dinv = (1.0 / deg).astype(np.float32)

    order = np.argsort(-indeg, kind="stable")  # rank -> node
    rank_of = np.empty(N, dtype=np.int64)
    rank_of[order] = np.arange(N)

    r_all = np.arange(NRANK)
    core_of_rank = (r_all % 1024) // 128
    l_of_rank = (r_all // 1024) * 128 + (r_all % 128)
    g_of_rank = core_of_rank * LOCAL + l_of_rank
    g_of_node = g_of_rank[rank_of]  # node -> table row

    indeg_rank = np.zeros(NRANK, dtype=np.int64)
    indeg_rank[:N] = indeg[order]
    K = np.maximum(indeg_rank[np.arange(T) * 1024], 1).astype(np.int64)
    off = np.concatenate([[0], np.cumsum(K)])
    totk = int(off[-1])

    # chunks of consecutive tiles, each <= MAXCOLS gather columns (+1 pad col)
    chunks = []
    t0 = 0
    acc = 0
    for t in range(T):
        if acc and acc + K[t] > MAXCOLS - 1:
            chunks.append((t0, t))
            t0 = t
            acc = 0
        acc += int(K[t])
    chunks.append((t0, T))

    # slot table [8, totk, 128] of table-row g values, init to pad rows
    padg = g_of_rank[N:NRANK]  # 176 all-zero rows (cores 6,7 tails)
    init = padg[np.arange(8 * totk * 128) % len(padg)]
    slot_g = init.reshape(8, totk, 128)

    dest_rank = rank_of[c_]
    sidx = np.argsort(dest_rank, kind="stable")
    dr = dest_rank[sidx]
    src_g = g_of_node[r_[sidx]]
    cnt = np.bincount(dr, minlength=NRANK)
    cum = np.concatenate([[0], np.cumsum(cnt)])
    within = np.arange(len(dr)) - cum[dr]
    t_d = dr // 1024
    c_d = (dr % 1024) // 128
    p_d = dr % 128
    colg = off[t_d] + within
    slot_g[c_d, colg, p_d] = src_g

    # final per-core index arrays with chunk pad columns appended
    wrapped = (slot_g - BASE).astype(np.int16)
    padcol = (padg[np.arange(128) % len(padg)] - BASE).astype(np.int16)  # >0
    per_core_idxs = []
    chunk_meta = []  # (idx_col_off, cols_ch, runs, gbuf_tile_offs)
    for c in range(NCORES):
        parts = []
        icol = 0
        for (a, b) in chunks:
            cols_ch = int(off[b] - off[a]) + 1
            parts.append(wrapped[c, off[a] : off[b], :])
            parts.append(padcol[None, :])
            if c == 0:
                # reduce runs: consecutive tiles with equal K
                runs = []
                t = a
                while t < b:
                    t2 = t
                    while t2 < b and K[t2] == K[t]:
                        t2 += 1
                    runs.append(
                        (int(off[t] - off[a]), t2 - t, int(K[t]), t)
                    )  # (col_off_in_chunk, ntiles, K, tile0)
                    t = t2
                chunk_meta.append((icol, cols_ch, runs))
            icol += cols_ch
        allcols = np.concatenate(parts, axis=0)  # [TOTC, 128]
        totc = allcols.shape[0]
        flat = allcols.reshape(-1)  # position j = colc*128 + p
        a16 = np.zeros((16, totc * 8), dtype=np.int16)
        j = np.arange(totc * 128)
        a16[j % 16, j // 16] = flat
        per_core_idxs.append(np.tile(a16, (8, 1)))
    totc_all = per_core_idxs[0].shape[1] // 8

    # per-core dinv [128, T]
    dinv_rank = np.zeros(NRANK, dtype=np.float32)
    dinv_rank[:N] = dinv[order]
    dpc = np.zeros((NCORES, P, T), dtype=np.float32)
    for c in range(NCORES):
        rr = (np.arange(T) * 1024)[None, :] + c * 128 + np.arange(P)[:, None]
        dpc[c] = dinv_rank[rr]

    # xTe [XROWS, NROWS]: col g holds x[node].T; row 770 = 1 for real cols
    xTe = np.zeros((XROWS, NROWS), dtype=np.float32)
    xTe[:IN_CH, g_of_node] = np.asarray(x, dtype=np.float32).T
    xTe[IN_CH, g_of_node] = 1.0

    layout = {
        "K": K,
        "chunks": chunks,
        "chunk_meta": chunk_meta,
        "totc": totc_all,
        "off": off,
    }
    return layout, per_core_idxs, dpc, xTe, g_of_node


def dma_gather_raw(nc, out_ap, in_ap, idxs_ap, num_idxs, elem_size, elem_step, queue_num=0):
    """bass dma_gather without the %256 elem-size assert (non-transpose, HBM
    source, multi-packet). Row stride (elem_step * 4B) must be %256 == 0."""
    gp = nc.gpsimd
    stride_bytes = elem_step * mybir.dt.size(in_ap.dtype)
    assert stride_bytes % 256 == 0 and stride_bytes // 256 < 256
    return gp.add_instruction(
        mybir.InstDMAGatherAnt(
            name=nc.get_next_instruction_name(),
            ins=[
                *gp.lower_ap_dma(in_ap, for_custom_bir_dma=True),
                gp.lower_ap(idxs_ap),
                gp.lower_val_access(gp.to_reg(num_idxs)),
            ],
            outs=[gp.lower_ap(out_ap)],
            transpose=False,
            num_idxs=num_idxs,
            elem_size=elem_size,
            stride_bytes_256=stride_bytes // 256,
            gen_mode=0,
            single_packet=False,
            queue_num=queue_num,
            sbuf_tokens_per_rank=0,
            sbuf_free_dim_per_rank=0,
            sbuf_free_dim_pad_per_rank=0,
            sbuf_byte_offset=0,
        )
    )


def build(layout):
    K = layout["K"]
    chunks = layout["chunks"]
    chunk_meta = layout["chunk_meta"]
    totc = layout["totc"]
    off = layout["off"]

    nc = bacc.Bacc("TRN2", num_devices=NCORES, debug=False, num_swdge_queues=4)

    xTe = nc.dram_tensor("xTe", [XROWS, LOCAL], F32R, kind="ExternalInput")
    w1 = nc.dram_tensor("w1", [XCH, NCHUNK_IN * 64], F32R, kind="ExternalInput")
    w2 = nc.dram_tensor("w2", [64, IN_CH], F32R, kind="ExternalInput")
    idxs = nc.dram_tensor("idxs", [P, totc * 8], I16, kind="ExternalInput")
    dinv_in = nc.dram_tensor("dinv", [P, T], F32, kind="ExternalInput")
    out = nc.dram_tensor("out", [LOCAL, IN_CH], F32, kind="ExternalOutput")

    z1loc = nc.dram_tensor("z1loc", [LOCAL, ROWE], F32)
    hloc = nc.dram_tensor("hloc", [LOCAL, ROWE], F32)
    z1g = nc.dram_tensor("z1g", [NROWS, ROWE], F32, addr_space="Shared")
    hg = nc.dram_tensor("hg", [NROWS, ROWE], F32, addr_space="Shared")

    stsizes = [512] * 12 + [128]  # node supertiles (6272 total)

    with tile.TileContext(nc) as tc:
        with (
            tc.tile_pool(name="persist", bufs=1) as pp,
            tc.tile_pool(name="xload", bufs=3) as xp,
            tc.tile_pool(name="gather", bufs=4) as gp_pool,
            tc.tile_pool(name="work", bufs=2) as wp,
            tc.tile_pool(name="outsb", bufs=3) as op_pool,
            tc.tile_pool(name="l1ps", bufs=2, space="PSUM") as l1ps,
            tc.tile_pool(name="trps", bufs=2, space="PSUM") as trps,
            tc.tile_pool(name="outps", bufs=2, space="PSUM") as outps,
        ):
            # ---- persistent loads ----
            w1_sb = pp.tile([XCH, NCHUNK_IN * 64], F32R)
            nc.sync.dma_start(w1_sb[:], w1[:])
            w2_sb = pp.tile([64, IN_CH], F32R)
            nc.sync.dma_start(w2_sb[:], w2[:])
            idxs_sb = pp.tile([P, totc * 8], I16)
            nc.sync.dma_start(idxs_sb[:], idxs[:])
            dinv_sb = pp.tile([P, T], F32)
            nc.sync.dma_start(dinv_sb[:], dinv_in[:])
            ident = pp.tile([P, P], F32)
            make_identity(nc, ident)

            z1r_sb = pp.tile([P, T * 64], F32)  # [z1 | r1+b1] per tile
            slotred = pp.tile([P, T * HID], F32)
            slotred2 = pp.tile([P, T * HID], F32)
            h_sb = pp.tile([P, T * HID], F32)
            tmp_sb = pp.tile([P, T * HID], F32)
            ag2h = pp.tile([P, T * 64], F32)  # [agg2 | h] per tile

            w1v = w1_sb[:].rearrange("p (k c) -> p k c", k=NCHUNK_IN)

            # ---- layer-1 matmul: z1|r1b = xTe_aug @ W1cat ----
            tglob = 0
            for st, stn in enumerate(stsizes):
                xsb = xp.tile([XCH, NCHUNK_IN, 512], F32R, tag="xsb")
                src = xTe.ap().rearrange("(k q) n -> q k n", q=XCH)[
                    :, :, st * 512 : st * 512 + stn
                ]
                nc.sync.dma_start(xsb[:, :, :stn], src)
                for tloc in range(stn // 128):
                    ps = l1ps.tile([P, 64], F32, space="PSUM")
                    for k in range(NCHUNK_IN):
                        nc.tensor.matmul(
                            out=ps[:],
                            lhsT=xsb[:, k, tloc * 128 : (tloc + 1) * 128],
                            rhs=w1v[:, k, :],
                            start=(k == 0),
                            stop=(k == NCHUNK_IN - 1),
                        )
                    nc.vector.tensor_copy(
                        z1r_sb[:, tglob * 64 : (tglob + 1) * 64], ps[:]
                    )
                    tglob += 1

            # ---- store z1 rows, AllGather ----
            z1v = z1r_sb[:].rearrange("p (t d) -> p t d", t=T)
            z1dst = z1loc.ap().rearrange("(t p) c -> p t c", p=P)[:, :, 0:HID]
            nc.sync.dma_start(z1dst, z1v[:, :, 0:HID])
            nc.gpsimd.collective_compute(
                "AllGather",
                mybir.AluOpType.bypass,
                replica_groups=[list(range(NCORES))],
                ins=[z1loc.ap().opt()],
                outs=[z1g.ap().opt()],
            )

            # ---- gather + reduce helper ----
            def gather_layer(table, dest_red, sems):
                for ci, (icol, cols_ch, runs) in enumerate(chunk_meta):
                    gbuf = gp_pool.tile([P, MAXCOLS, HID], F32, tag="gbuf")
                    sem = sems[ci]
                    with tc.tile_critical(no_gpsimd_drain=True):
                        dma_gather_raw(
                            nc,
                            gbuf[:, :cols_ch, :],
                            table[BASE:, :],
                            idxs_sb[:, icol * 8 : (icol + cols_ch) * 8],
                            num_idxs=cols_ch * 128,
                            elem_size=HID,
                            elem_step=ROWE,
                            queue_num=ci % 4,
                        ).then_inc(sem, 16)
                    with tc.tile_critical():
                        nc.vector.wait_ge(sem, 16)
                        for (coff, nt, kk, t0) in runs:
                            inv = gbuf[:, coff : coff + nt * kk, :].rearrange(
                                "p (t k) c -> p t c k", k=kk
                            )
                            nc.vector.tensor_reduce(
                                out=dest_red[:, t0 * HID : (t0 + nt) * HID],
                                in_=inv,
                                axis=mybir.AxisListType.X,
                                op=mybir.AluOpType.add,
                            )

            sems1 = [nc.alloc_semaphore(f"g1_{i}") for i in range(len(chunk_meta))]
            gather_layer(z1g, slotred, sems1)

            # ---- h = relu(dinv*(slotred + 2*z1) + r1b) ----
            dinv_b = dinv_sb[:].to_broadcast([P, T, HID])
            sr_v = slotred[:].rearrange("p (t c) -> p t c", t=T)
            tmp_v = tmp_sb[:].rearrange("p (t c) -> p t c", t=T)
            h_v = h_sb[:].rearrange("p (t c) -> p t c", t=T)
            nc.vector.tensor_scalar(
                out=tmp_v, in0=z1v[:, :, 0:HID], scalar1=2.0, scalar2=None,
                op0=mybir.AluOpType.mult,
            )
            nc.vector.tensor_tensor(
                out=tmp_sb[:], in0=tmp_sb[:], in1=slotred[:], op=mybir.AluOpType.add
            )
            nc.vector.tensor_tensor(
                out=tmp_v, in0=tmp_v, in1=dinv_b, op=mybir.AluOpType.mult
            )
            nc.vector.tensor_tensor(
                out=tmp_v, in0=tmp_v, in1=z1v[:, :, HID:64],
                op=mybir.AluOpType.add,
            )
            nc.vector.tensor_scalar(
                out=h_sb[:], in0=tmp_sb[:], scalar1=0.0, scalar2=None,
                op0=mybir.AluOpType.max,
            )

            ag2h_v = ag2h[:].rearrange("p (t d) -> p t d", t=T)
            nc.vector.tensor_copy(ag2h_v[:, :, HID:64], h_v)

            # ---- store h rows, AllGather ----
            hdst = hloc.ap().rearrange("(t p) c -> p t c", p=P)[:, :, 0:HID]
            nc.sync.dma_start(hdst, h_v)
            nc.gpsimd.collective_compute(
                "AllGather",
                mybir.AluOpType.bypass,
                replica_groups=[list(range(NCORES))],
                ins=[hloc.ap().opt()],
                outs=[hg.ap().opt()],
            )

            sems2 = [nc.alloc_semaphore(f"g2_{i}") for i in range(len(chunk_meta))]
            gather_layer(hg, slotred2, sems2)

            # ---- agg2 = dinv*(slotred2 + 2*h) -> ag2h[:, :, 0:HID] ----
            sr2_v = slotred2[:].rearrange("p (t c) -> p t c", t=T)
            nc.vector.tensor_scalar(
                out=tmp_sb[:], in0=h_sb[:], scalar1=2.0, scalar2=None,
                op0=mybir.AluOpType.mult,
            )
            nc.vector.tensor_tensor(
                out=tmp_sb[:], in0=tmp_sb[:], in1=slotred2[:], op=mybir.AluOpType.add
            )
            nc.vector.tensor_tensor(
                out=ag2h_v[:, :, 0:HID], in0=tmp_v, in1=dinv_b,
                op=mybir.AluOpType.mult,
            )

            # ---- per tile: transpose -> catT, matmul, copy out, DMA ----
            for t in range(T):
                tp = trps.tile([64, P], F32, space="PSUM")
                nc.tensor.transpose(
                    out=tp[:], in_=ag2h[:, t * 64 : (t + 1) * 64], identity=ident[:]
                )
                catT = wp.tile([64, P], F32R, tag="catT")
                nc.vector.tensor_copy(catT[:], tp[:])
                pso = outps.tile([P, IN_CH], F32, space="PSUM")
                nc.tensor.matmul(
                    out=pso[:, 0:512], lhsT=catT[:], rhs=w2_sb[:, 0:512],
                    start=True, stop=True,
                )
                nc.tensor.matmul(
                    out=pso[:, 512:IN_CH], lhsT=catT[:], rhs=w2_sb[:, 512:IN_CH],
                    start=True, stop=True,
                )
                osb = op_pool.tile([P, IN_CH], F32, tag="osb")
                if t % 2 == 0:
                    nc.vector.tensor_copy(osb[:], pso[:])
                else:
                    nc.scalar.copy(osb[:], pso[:])
                nc.sync.dma_start(out[t * P : (t + 1) * P, :], osb[:])

    nc.compile()
    return nc


_CACHE = {}


def kernel(x, edge_index, W_out1, b_out1, W_root1, W_out2, b_out2, W_root2):
    x = np.asarray(x, dtype=np.float32)
    layout, per_core_idxs, dpc, xTe, g_of_node = _host_prep(x, edge_index)

    # weight tensors
    w1cat = np.zeros((XROWS, 64), dtype=np.float32)
    w1cat[:IN_CH, :HID] = np.asarray(W_out1, dtype=np.float32).T
    w1cat[:IN_CH, HID:] = np.asarray(W_root1, dtype=np.float32).T
    w1cat[IN_CH, HID:] = np.asarray(b_out1, dtype=np.float32)
    # rearrange to [XCH, NCHUNK_IN*64]: [p, k*64+c] = w1cat[k*XCH+p, c]
    w1_dev = np.ascontiguousarray(
        w1cat.reshape(NCHUNK_IN, XCH, 64).transpose(1, 0, 2).reshape(XCH, -1)
    )
    w2_dev = np.concatenate(
        [np.asarray(W_out2, dtype=np.float32).T, np.asarray(W_root2, dtype=np.float32).T],
        axis=0,
    )  # [64, 770]

    key = (tuple(int(k) for k in layout["K"]), tuple(layout["chunks"]))
    if key not in _CACHE:
        _CACHE[key] = build(layout)
    nc = _CACHE[key]

    in_maps = []
    for c in range(NCORES):
        in_maps.append(
            {
                "xTe": np.ascontiguousarray(xTe[:, c * LOCAL : (c + 1) * LOCAL]),
                "w1": w1_dev,
                "w2": np.ascontiguousarray(w2_dev),
                "idxs": per_core_idxs[c],
                "dinv": dpc[c],
            }
        )

    trace = os.environ.get("GCN_TRACE", "0") == "1"
    res = run_bass_kernel_spmd(
        nc, in_maps, core_ids=list(range(NCORES)), trace=trace
    )
    if trace and res.exec_time_ns is not None:
        print(f"HW exec time: {res.exec_time_ns} ns")
        kernel.last_exec_time_ns = res.exec_time_ns

    out = np.empty((N, IN_CH), dtype=np.float32)
    # table row g -> node
    node_of_g = np.full(NROWS, -1, dtype=np.int64)
    node_of_g[g_of_node] = np.arange(N)
    for c in range(NCORES):
        gs = np.arange(c * LOCAL, (c + 1) * LOCAL)
        nodes = node_of_g[gs]
        m = nodes >= 0
        out[nodes[m]] = res.results[c]["out"][m]
    out += np.asarray(b_out2, dtype=np.float32)[None, :]
    return out


# revision 5
# speedup vs baseline: 1.1352x; 1.0549x over previous
"""ClusterGCN 2-layer kernel for 8 Trainium2 NeuronCores.

Strategy:
 - Exploit linearity: project x (770ch) down to 32ch FIRST (z1 = x @ W_out1.T),
   then message-pass on 32-dim vectors (24x less gather traffic).
 - Edge weight = deg_inv[dest] (uniform per destination) => aggregate raw
   neighbor sums, scale once per destination.
 - Nodes degree-sorted and dealt across 8 cores; per-destination padded slot
   lists (gather indices) shared by both layers.
 - Device: z1|r1 via PE matmul, AllGather z1 (row-major table, 256B stride),
   one dma_gather per chunk of destination tiles (int16 signed wrapped
   indices with +32768-row base), strided middle-axis tensor_reduce,
   elementwise assembly of h, AllGather h, same gathers again, PE transpose,
   final f32r matmul to [128, 770] output tiles.
"""

import os
import sys
import types

import numpy as np

# ---------------------------------------------------------------------------
# environment shims (axon NTFF hook + no artifact bucket)
# ---------------------------------------------------------------------------
if "antenv.axon_hooks" not in sys.modules:
    _mod = types.ModuleType("antenv.axon_hooks")
    _hook_store = [None]
    _mod.set_axon_ntff_profile_hook = lambda h: _hook_store.__setitem__(0, h)
    _mod.get_axon_ntff_profile_hook = lambda: _hook_store[0]
    try:
        import antenv

        antenv.axon_hooks = _mod
        sys.modules["antenv.axon_hooks"] = _mod
        from trn_agent_boot.trn_boot import _ntff_profile_via_ctypes

        _mod.set_axon_ntff_profile_hook(
            _ntff_profile_via_ctypes("/opt/axon/libaxon_pjrt.so")
        )
    except Exception:
        pass

import concourse.bacc as bacc
import concourse.bass as bass
import concourse.bass_utils as bass_utils
import concourse.mybir as mybir
import concourse.tile as tile
from concourse.bass_utils import run_bass_kernel_spmd
from concourse.masks import make_identity

bass_utils.upload_artifacts = lambda tmpdir: tmpdir

# ---------------------------------------------------------------------------
# problem constants (hardcoded per the harness contract)
# ---------------------------------------------------------------------------
N = 50000
E = 400000
IN_CH = 770
HID = 32
DIAG_LAMBDA = 1.0
NCORES = 8
P = 128
T = 49  # destination tiles per core
LOCAL = T * P  # 6272 rows per core block
NRANK = T * 1024  # 50176 ranks total
NROWS = NCORES * LOCAL  # 50176 gather-table rows
BASE = 32768  # gather base-row offset (signed int16 wrap)
ROWE = 64  # table row stride in f32 elems (256B)
XCH = 112  # input-channel chunk (7 chunks x 112 = 784)
NCHUNK_IN = 7
XROWS = NCHUNK_IN * XCH  # 784 = 770 + ones row + 13 zero rows
MAXCOLS = 104  # max gather columns per dma_gather chunk

F32 = mybir.dt.float32
BF16 = mybir.dt.bfloat16
F32R = mybir.dt.float32r
I16 = mybir.dt.int16


def _host_prep(x, edge_index):
    """Degree stats, node permutation, gather slot tables, xTe."""
    row = np.asarray(edge_index[0], dtype=np.int64)
    col = np.asarray(edge_index[1], dtype=np.int64)
    ns = row != col
    r_, c_ = row[ns], col[ns]
    indeg = np.bincount(c_, minlength=N)
    deg = (indeg + 1).astype(np.float64)
    dinv = (1.0 / deg).astype(np.float32)

    order = np.argsort(-indeg, kind="stable")  # rank -> node
    rank_of = np.empty(N, dtype=np.int64)
    rank_of[order] = np.arange(N)

    r_all = np.arange(NRANK)
    core_of_rank = (r_all % 1024) // 128
    l_of_rank = (r_all // 1024) * 128 + (r_all % 128)
    g_of_rank = core_of_rank * LOCAL + l_of_rank
    g_of_node = g_of_rank[rank_of]  # node -> table row

    indeg_rank = np.zeros(NRANK, dtype=np.int64)
    indeg_rank[:N] = indeg[order]
    K = np.maximum(indeg_rank[np.arange(T) * 1024], 1).astype(np.int64)
    off = np.concatenate([[0], np.cumsum(K)])
    totk = int(off[-1])

    # chunks of consecutive tiles, each <= MAXCOLS gather columns (+1 pad col)
    chunks = []
    t0 = 0
    acc = 0
    for t in range(T):
        if acc and acc + K[t] > MAXCOLS - 1:
            chunks.append((t0, t))
            t0 = t
            acc = 0
        acc += int(K[t])
    chunks.append((t0, T))

    # slot table [8, totk, 128] of table-row g values, init to pad rows
    padg = g_of_rank[N:NRANK]  # 176 all-zero rows (cores 6,7 tails)
    init = padg[np.arange(8 * totk * 128) % len(padg)]
    slot_g = init.reshape(8, totk, 128)

    dest_rank = rank_of[c_]
    sidx = np.argsort(dest_rank, kind="stable")
    dr = dest_rank[sidx]
    src_g = g_of_node[r_[sidx]]
    cnt = np.bincount(dr, minlength=NRANK)
    cum = np.concatenate([[0], np.cumsum(cnt)])
    within = np.arange(len(dr)) - cum[dr]
    t_d = dr // 1024
    c_d = (dr % 1024) // 128
    p_d = dr % 128
    colg = off[t_d] + within
    slot_g[c_d, colg, p_d] = src_g

    # final per-core index arrays with chunk pad columns appended
    wrapped = (slot_g - BASE).astype(np.int16)
    padcol = (padg[np.arange(128) % len(padg)] - BASE).astype(np.int16)  # >0
    per_core_idxs = []
    chunk_meta = []  # (idx_col_off, cols_ch, runs, gbuf_tile_offs)
    for c in range(NCORES):
        parts = []
        icol = 0
        for (a, b) in chunks:
            cols_ch = int(off[b] - off[a]) + 1
            parts.append(wrapped[c, off[a] : off[b], :])
            parts.append(padcol[None, :])
            if c == 0:
                # reduce runs: consecutive tiles with equal K
                runs = []
                t = a
                while t < b:
                    t2 = t
                    while t2 < b and K[t2] == K[t]:
                        t2 += 1
                    runs.append(
                        (int(off[t] - off[a]), t2 - t, int(K[t]), t)
                    )  # (col_off_in_chunk, ntiles, K, tile0)
                    t = t2
                chunk_meta.append((icol, cols_ch, runs))
            icol += cols_ch
        allcols = np.concatenate(parts, axis=0)  # [TOTC, 128]
        totc = allcols.shape[0]
        flat = allcols.reshape(-1)  # position j = colc*128 + p
        a16 = np.zeros((16, totc * 8), dtype=np.int16)
        j = np.arange(totc * 128)
        a16[j % 16, j // 16] = flat
        per_core_idxs.append(np.tile(a16, (8, 1)))
    totc_all = per_core_idxs[0].shape[1] // 8

    # per-core dinv [128, T]
    dinv_rank = np.zeros(NRANK, dtype=np.float32)
    dinv_rank[:N] = dinv[order]
    dpc = np.zeros((NCORES, P, T), dtype=np.float32)
    for c in range(NCORES):
        rr = (np.arange(T) * 1024)[None, :] + c * 128 + np.arange(P)[:, None]
        dpc[c] = dinv_rank[rr]

    # xTe [XROWS, NROWS]: col g holds x[node].T; row 770 = 1 for real cols
    xTe = np.zeros((XROWS, NROWS), dtype=np.float32)
    xTe[:IN_CH, g_of_node] = np.asarray(x, dtype=np.float32).T
    xTe[IN_CH, g_of_node] = 1.0

    layout = {
        "K": K,
        "chunks": chunks,
        "chunk_meta": chunk_meta,
        "totc": totc_all,
        "off": off,
    }
    return layout, per_core_idxs, dpc, xTe, g_of_node


def dma_gather_raw(nc, out_ap, in_ap, idxs_ap, num_idxs, elem_size, elem_step, queue_num=0):
    """bass dma_gather without the %256 elem-size assert (non-transpose, HBM
    source, multi-packet). Row stride (elem_step * 4B) must be %256 == 0."""
    gp = nc.gpsimd
    stride_bytes = elem_step * mybir.dt.size(in_ap.dtype)
    assert stride_bytes % 256 == 0 and stride_bytes // 256 < 256
    return gp.add_instruction(
        mybir.InstDMAGatherAnt(
            name=nc.get_next_instruction_name(),
            ins=[
                *gp.lower_ap_dma(in_ap, for_custom_bir_dma=True),
                gp.lower_ap(idxs_ap),
                gp.lower_val_access(gp.to_reg(num_idxs)),
            ],
            outs=[gp.lower_ap(out_ap)],
            transpose=False,
            num_idxs=num_idxs,
            elem_size=elem_size,
            stride_bytes_256=stride_bytes // 256,
            gen_mode=0,
            single_packet=False,
            queue_num=queue_num,
            sbuf_tokens_per_rank=0,
            sbuf_free_dim_per_rank=0,
            sbuf_free_dim_pad_per_rank=0,
            sbuf_byte_offset=0,
        )
    )


def build(layout):
    K = layout["K"]
    chunks = layout["chunks"]
    chunk_meta = layout["chunk_meta"]
    totc = layout["totc"]
    off = layout["off"]

    nc = bacc.Bacc("TRN2", num_devices=NCORES, debug=False, num_swdge_queues=4)

    xTe = nc.dram_tensor("xTe", [XROWS, LOCAL], BF16, kind="ExternalInput")
    w1 = nc.dram_tensor("w1", [XCH, NCHUNK_IN * 64], BF16, kind="ExternalInput")
    w2 = nc.dram_tensor("w2", [64, IN_CH], F32R, kind="ExternalInput")
    idxs = nc.dram_tensor("idxs", [P, totc * 8], I16, kind="ExternalInput")
    dinv_in = nc.dram_tensor("dinv", [P, T], F32, kind="ExternalInput")
    out = nc.dram_tensor("out", [LOCAL, IN_CH], F32, kind="ExternalOutput")

    z1loc = nc.dram_tensor("z1loc", [LOCAL, ROWE], F32)
    hloc = nc.dram_tensor("hloc", [LOCAL, ROWE], F32)
    z1g = nc.dram_tensor("z1g", [NROWS, ROWE], F32, addr_space="Shared")
    hg = nc.dram_tensor("hg", [NROWS, ROWE], F32, addr_space="Shared")

    stsizes = [512] * 12 + [128]  # node supertiles (6272 total)

    with tile.TileContext(nc) as tc:
        with (
            tc.tile_pool(name="persist", bufs=1) as pp,
            tc.tile_pool(name="xload", bufs=3) as xp,
            tc.tile_pool(name="gather", bufs=4) as gp_pool,
            tc.tile_pool(name="work", bufs=2) as wp,
            tc.tile_pool(name="outsb", bufs=3) as op_pool,
            tc.tile_pool(name="l1ps", bufs=2, space="PSUM") as l1ps,
            tc.tile_pool(name="trps", bufs=2, space="PSUM") as trps,
            tc.tile_pool(name="outps", bufs=2, space="PSUM") as outps,
        ):
            # ---- persistent loads ----
            w1_sb = pp.tile([XCH, NCHUNK_IN * 64], BF16)
            nc.sync.dma_start(w1_sb[:], w1[:])
            w2_sb = pp.tile([64, IN_CH], F32R)
            nc.sync.dma_start(w2_sb[:], w2[:])
            idxs_sb = pp.tile([P, totc * 8], I16)
            nc.sync.dma_start(idxs_sb[:], idxs[:])
            dinv_sb = pp.tile([P, T], F32)
            nc.sync.dma_start(dinv_sb[:], dinv_in[:])
            ident = pp.tile([P, P], F32)
            make_identity(nc, ident)

            z1r_sb = pp.tile([P, T * 64], F32)  # [z1 | r1+b1] per tile
            slotred = pp.tile([P, T * HID], F32)
            slotred2 = pp.tile([P, T * HID], F32)
            h_sb = pp.tile([P, T * HID], F32)
            tmp_sb = pp.tile([P, T * HID], F32)
            ag2h = pp.tile([P, T * 64], F32)  # [agg2 | h] per tile

            w1v = w1_sb[:].rearrange("p (k c) -> p k c", k=NCHUNK_IN)

            # ---- layer-1 matmul: z1|r1b = xTe_aug @ W1cat ----
            tglob = 0
            for st, stn in enumerate(stsizes):
                xsb = xp.tile([XCH, NCHUNK_IN, 512], BF16, tag="xsb")
                src = xTe.ap().rearrange("(k q) n -> q k n", q=XCH)[
                    :, :, st * 512 : st * 512 + stn
                ]
                nc.sync.dma_start(xsb[:, :, :stn], src)
                for tloc in range(stn // 128):
                    ps = l1ps.tile([P, 64], F32, space="PSUM")
                    for k in range(NCHUNK_IN):
                        nc.tensor.matmul(
                            out=ps[:],
                            lhsT=xsb[:, k, tloc * 128 : (tloc + 1) * 128],
                            rhs=w1v[:, k, :],
                            start=(k == 0),
                            stop=(k == NCHUNK_IN - 1),
                        )
                    nc.vector.tensor_copy(
                        z1r_sb[:, tglob * 64 : (tglob + 1) * 64], ps[:]
                    )
                    tglob += 1

            # ---- store z1 rows, AllGather ----
            z1v = z1r_sb[:].rearrange("p (t d) -> p t d", t=T)
            z1dst = z1loc.ap().rearrange("(t p) c -> p t c", p=P)[:, :, 0:HID]
            nc.sync.dma_start(z1dst, z1v[:, :, 0:HID])
            nc.gpsimd.collective_compute(
                "AllGather",
                mybir.AluOpType.bypass,
                replica_groups=[list(range(NCORES))],
                ins=[z1loc.ap().opt()],
                outs=[z1g.ap().opt()],
            )

            # ---- gather + reduce helper ----
            def gather_layer(table, dest_red, sems, chunk_cb=None):
                for ci, (icol, cols_ch, runs) in enumerate(chunk_meta):
                    gbuf = gp_pool.tile([P, MAXCOLS, HID], F32, tag="gbuf")
                    sem = sems[ci]
                    with tc.tile_critical(no_gpsimd_drain=True):
                        dma_gather_raw(
                            nc,
                            gbuf[:, :cols_ch, :],
                            table[BASE:, :],
                            idxs_sb[:, icol * 8 : (icol + cols_ch) * 8],
                            num_idxs=cols_ch * 128,
                            elem_size=HID,
                            elem_step=ROWE,
                            queue_num=ci % 4,
                        ).then_inc(sem, 16)
                    with tc.tile_critical():
                        nc.vector.wait_ge(sem, 16)
                        for (coff, nt, kk, t0) in runs:
                            inv = gbuf[:, coff : coff + nt * kk, :].rearrange(
                                "p (t k) c -> p t c k", k=kk
                            )
                            nc.vector.tensor_reduce(
                                out=dest_red[:, t0 * HID : (t0 + nt) * HID],
                                in_=inv,
                                axis=mybir.AxisListType.X,
                                op=mybir.AluOpType.add,
                            )
                    if chunk_cb is not None:
                        chunk_cb(ci)

            sems1 = [nc.alloc_semaphore(f"g1_{i}") for i in range(len(chunk_meta))]
            gather_layer(z1g, slotred, sems1)

            # ---- h = relu(dinv*(slotred + 2*z1) + r1b) ----
            dinv_b = dinv_sb[:].to_broadcast([P, T, HID])
            sr_v = slotred[:].rearrange("p (t c) -> p t c", t=T)
            tmp_v = tmp_sb[:].rearrange("p (t c) -> p t c", t=T)
            h_v = h_sb[:].rearrange("p (t c) -> p t c", t=T)
            nc.vector.tensor_scalar(
                out=tmp_v, in0=z1v[:, :, 0:HID], scalar1=2.0, scalar2=None,
                op0=mybir.AluOpType.mult,
            )
            nc.vector.tensor_tensor(
                out=tmp_sb[:], in0=tmp_sb[:], in1=slotred[:], op=mybir.AluOpType.add
            )
            nc.vector.tensor_tensor(
                out=tmp_v, in0=tmp_v, in1=dinv_b, op=mybir.AluOpType.mult
            )
            nc.vector.tensor_tensor(
                out=tmp_v, in0=tmp_v, in1=z1v[:, :, HID:64],
                op=mybir.AluOpType.add,
            )
            nc.vector.tensor_scalar(
                out=h_sb[:], in0=tmp_sb[:], scalar1=0.0, scalar2=None,
                op0=mybir.AluOpType.max,
            )

            ag2h_v = ag2h[:].rearrange("p (t d) -> p t d", t=T)
            nc.vector.tensor_copy(ag2h_v[:, :, HID:64], h_v)

            # ---- store h rows, AllGather ----
            hdst = hloc.ap().rearrange("(t p) c -> p t c", p=P)[:, :, 0:HID]
            nc.sync.dma_start(hdst, h_v)
            nc.gpsimd.collective_compute(
                "AllGather",
                mybir.AluOpType.bypass,
                replica_groups=[list(range(NCORES))],
                ins=[hloc.ap().opt()],
                outs=[hg.ap().opt()],
            )

            # ---- L2: per-chunk assembly + output pipeline ----
            def l2_chunk(ci):
                a, b = chunks[ci]
                nt = b - a
                sl = slice(a * HID, b * HID)
                tv = tmp_sb[:, sl].rearrange("p (t c) -> p t c", t=nt)
                nc.vector.tensor_scalar(
                    out=tmp_sb[:, sl], in0=h_sb[:, sl], scalar1=2.0, scalar2=None,
                    op0=mybir.AluOpType.mult,
                )
                nc.vector.tensor_tensor(
                    out=tmp_sb[:, sl], in0=tmp_sb[:, sl], in1=slotred2[:, sl],
                    op=mybir.AluOpType.add,
                )
                nc.vector.tensor_tensor(
                    out=ag2h_v[:, a:b, 0:HID], in0=tv,
                    in1=dinv_sb[:, a:b].to_broadcast([P, nt, HID]),
                    op=mybir.AluOpType.mult,
                )
                for t in range(a, b):
                    tp = trps.tile([64, P], F32, space="PSUM")
                    nc.tensor.transpose(
                        out=tp[:], in_=ag2h[:, t * 64 : (t + 1) * 64],
                        identity=ident[:],
                    )
                    catT = wp.tile([64, P], F32R, tag="catT")
                    nc.vector.tensor_copy(catT[:], tp[:])
                    pso = outps.tile([P, IN_CH], F32, space="PSUM")
                    nc.tensor.matmul(
                        out=pso[:, 0:512], lhsT=catT[:], rhs=w2_sb[:, 0:512],
                        start=True, stop=True,
                    )
                    nc.tensor.matmul(
                        out=pso[:, 512:IN_CH], lhsT=catT[:], rhs=w2_sb[:, 512:IN_CH],
                        start=True, stop=True,
                    )
                    osb = op_pool.tile([P, IN_CH], F32, tag="osb")
                    if t % 2 == 0:
                        nc.vector.tensor_copy(osb[:], pso[:])
                    else:
                        nc.scalar.copy(osb[:], pso[:])
                    nc.sync.dma_start(out[t * P : (t + 1) * P, :], osb[:])

            sems2 = [nc.alloc_semaphore(f"g2_{i}") for i in range(len(chunk_meta))]
            gather_layer(hg, slotred2, sems2, chunk_cb=l2_chunk)

    nc.compile()
    return nc


_CACHE = {}


def kernel(x, edge_index, W_out1, b_out1, W_root1, W_out2, b_out2, W_root2):
    x = np.asarray(x, dtype=np.float32)
    layout, per_core_idxs, dpc, xTe, g_of_node = _host_prep(x, edge_index)

    # weight tensors
    w1cat = np.zeros((XROWS, 64), dtype=np.float32)
    w1cat[:IN_CH, :HID] = np.asarray(W_out1, dtype=np.float32).T
    w1cat[:IN_CH, HID:] = np.asarray(W_root1, dtype=np.float32).T
    w1cat[IN_CH, HID:] = np.asarray(b_out1, dtype=np.float32)
    # rearrange to [XCH, NCHUNK_IN*64]: [p, k*64+c] = w1cat[k*XCH+p, c]
    import ml_dtypes

    w1_dev = np.ascontiguousarray(
        w1cat.reshape(NCHUNK_IN, XCH, 64).transpose(1, 0, 2).reshape(XCH, -1)
    ).astype(ml_dtypes.bfloat16)
    w2_dev = np.concatenate(
        [np.asarray(W_out2, dtype=np.float32).T, np.asarray(W_root2, dtype=np.float32).T],
        axis=0,
    )  # [64, 770]

    key = (tuple(int(k) for k in layout["K"]), tuple(layout["chunks"]))
    if key not in _CACHE:
        _CACHE[key] = build(layout)
    nc = _CACHE[key]

    in_maps = []
    for c in range(NCORES):
        in_maps.append(
            {
                "xTe": np.ascontiguousarray(xTe[:, c * LOCAL : (c + 1) * LOCAL]).astype(ml_dtypes.bfloat16),
                "w1": w1_dev,
                "w2": np.ascontiguousarray(w2_dev),
                "idxs": per_core_idxs[c],
                "dinv": dpc[c],
            }
        )

    trace = os.environ.get("GCN_TRACE", "0") == "1"
    res = run_bass_kernel_spmd(
        nc, in_maps, core_ids=list(range(NCORES)), trace=trace
    )
    if trace and res.exec_time_ns is not None:
        print(f"HW exec time: {res.exec_time_ns} ns")
        kernel.last_exec_time_ns = res.exec_time_ns

    out = np.empty((N, IN_CH), dtype=np.float32)
    # table row g -> node
    node_of_g = np.full(NROWS, -1, dtype=np.int64)
    node_of_g[g_of_node] = np.arange(N)
    for c in range(NCORES):
        gs = np.arange(c * LOCAL, (c + 1) * LOCAL)
        nodes = node_of_g[gs]
        m = nodes >= 0
        out[nodes[m]] = res.results[c]["out"][m]
    out += np.asarray(b_out2, dtype=np.float32)[None, :]
    return out


# revision 6
# speedup vs baseline: 1.2659x; 1.1152x over previous
"""ClusterGCN 2-layer kernel for 8 Trainium2 NeuronCores.

Strategy:
 - Exploit linearity: project x (770ch) down to 32ch FIRST (z1 = x @ W_out1.T),
   then message-pass on 32-dim vectors (24x less gather traffic).
 - Edge weight = deg_inv[dest] (uniform per destination) => aggregate raw
   neighbor sums, scale once per destination.
 - Nodes degree-sorted and dealt across 8 cores; per-destination padded slot
   lists (gather indices) shared by both layers.
 - Device: z1|r1 via PE matmul, AllGather z1 (row-major table, 256B stride),
   one dma_gather per chunk of destination tiles (int16 signed wrapped
   indices with +32768-row base), strided middle-axis tensor_reduce,
   elementwise assembly of h, AllGather h, same gathers again, PE transpose,
   final f32r matmul to [128, 770] output tiles.
"""

import os
import sys
import types

import numpy as np

# ---------------------------------------------------------------------------
# environment shims (axon NTFF hook + no artifact bucket)
# ---------------------------------------------------------------------------
if "antenv.axon_hooks" not in sys.modules:
    _mod = types.ModuleType("antenv.axon_hooks")
    _hook_store = [None]
    _mod.set_axon_ntff_profile_hook = lambda h: _hook_store.__setitem__(0, h)
    _mod.get_axon_ntff_profile_hook = lambda: _hook_store[0]
    try:
        import antenv

        antenv.axon_hooks = _mod
        sys.modules["antenv.axon_hooks"] = _mod
        from trn_agent_boot.trn_boot import _ntff_profile_via_ctypes

        _mod.set_axon_ntff_profile_hook(
            _ntff_profile_via_ctypes("/opt/axon/libaxon_pjrt.so")
        )
    except Exception:
        pass

import concourse.bacc as bacc
import concourse.bass as bass
import concourse.bass_utils as bass_utils
import concourse.mybir as mybir
import concourse.tile as tile
from concourse.bass_utils import run_bass_kernel_spmd
from concourse.masks import make_identity

bass_utils.upload_artifacts = lambda tmpdir: tmpdir

# ---------------------------------------------------------------------------
# problem constants (hardcoded per the harness contract)
# ---------------------------------------------------------------------------
N = 50000
E = 400000
IN_CH = 770
HID = 32
DIAG_LAMBDA = 1.0
NCORES = 8
P = 128
T = 49  # destination tiles per core
LOCAL = T * P  # 6272 rows per core block
NRANK = T * 1024  # 50176 ranks total
NROWS = NCORES * LOCAL  # 50176 gather-table rows
BASE = 32768  # gather base-row offset (signed int16 wrap)
ROWE = 64  # table row stride in f32 elems (256B)
XCH = 112  # input-channel chunk (7 chunks x 112 = 784)
NCHUNK_IN = 7
XROWS = NCHUNK_IN * XCH  # 784 = 770 + ones row + 13 zero rows
MAXCOLS = 104  # max gather columns per dma_gather chunk

F32 = mybir.dt.float32
BF16 = mybir.dt.bfloat16
F32R = mybir.dt.float32r
I16 = mybir.dt.int16


def _host_prep(x, edge_index):
    """Degree stats, node permutation, gather slot tables, xTe."""
    row = np.asarray(edge_index[0], dtype=np.int64)
    col = np.asarray(edge_index[1], dtype=np.int64)
    ns = row != col
    r_, c_ = row[ns], col[ns]
    indeg = np.bincount(c_, minlength=N)
    deg = (indeg + 1).astype(np.float64)
    dinv = (1.0 / deg).astype(np.float32)

    order = np.argsort(-indeg, kind="stable")  # rank -> node
    rank_of = np.empty(N, dtype=np.int64)
    rank_of[order] = np.arange(N)

    r_all = np.arange(NRANK)
    core_of_rank = (r_all % 1024) // 128
    l_of_rank = (r_all // 1024) * 128 + (r_all % 128)
    g_of_rank = core_of_rank * LOCAL + l_of_rank
    g_of_node = g_of_rank[rank_of]  # node -> table row

    indeg_rank = np.zeros(NRANK, dtype=np.int64)
    indeg_rank[:N] = indeg[order]
    K = np.maximum(indeg_rank[np.arange(T) * 1024], 1).astype(np.int64)
    off = np.concatenate([[0], np.cumsum(K)])
    totk = int(off[-1])

    # chunks of consecutive tiles, each <= MAXCOLS gather columns (+1 pad col)
    chunks = []
    t0 = 0
    acc = 0
    for t in range(T):
        if acc and acc + K[t] > MAXCOLS - 1:
            chunks.append((t0, t))
            t0 = t
            acc = 0
        acc += int(K[t])
    chunks.append((t0, T))

    # slot table [8, totk, 128] of table-row g values, init to pad rows
    padg = g_of_rank[N:NRANK]  # 176 all-zero rows (cores 6,7 tails)
    init = padg[np.arange(8 * totk * 128) % len(padg)]
    slot_g = init.reshape(8, totk, 128)

    dest_rank = rank_of[c_]
    sidx = np.argsort(dest_rank, kind="stable")
    dr = dest_rank[sidx]
    src_g = g_of_node[r_[sidx]]
    cnt = np.bincount(dr, minlength=NRANK)
    cum = np.concatenate([[0], np.cumsum(cnt)])
    within = np.arange(len(dr)) - cum[dr]
    t_d = dr // 1024
    c_d = (dr % 1024) // 128
    p_d = dr % 128
    colg = off[t_d] + within
    slot_g[c_d, colg, p_d] = src_g

    # final per-core index arrays with chunk pad columns appended
    wrapped = (slot_g - BASE).astype(np.int16)
    padcol = (padg[np.arange(128) % len(padg)] - BASE).astype(np.int16)  # >0
    per_core_idxs = []
    chunk_meta = []  # (idx_col_off, cols_ch, runs, gbuf_tile_offs)
    for c in range(NCORES):
        parts = []
        icol = 0
        for (a, b) in chunks:
            cols_ch = int(off[b] - off[a]) + 1
            parts.append(wrapped[c, off[a] : off[b], :])
            parts.append(padcol[None, :])
            if c == 0:
                # reduce runs: consecutive tiles with equal K
                runs = []
                t = a
                while t < b:
                    t2 = t
                    while t2 < b and K[t2] == K[t]:
                        t2 += 1
                    runs.append(
                        (int(off[t] - off[a]), t2 - t, int(K[t]), t)
                    )  # (col_off_in_chunk, ntiles, K, tile0)
                    t = t2
                chunk_meta.append((icol, cols_ch, runs))
            icol += cols_ch
        allcols = np.concatenate(parts, axis=0)  # [TOTC, 128]
        totc = allcols.shape[0]
        flat = allcols.reshape(-1)  # position j = colc*128 + p
        a16 = np.zeros((16, totc * 8), dtype=np.int16)
        j = np.arange(totc * 128)
        a16[j % 16, j // 16] = flat
        per_core_idxs.append(np.tile(a16, (8, 1)))
    totc_all = per_core_idxs[0].shape[1] // 8

    # per-core dinv [128, T]
    dinv_rank = np.zeros(NRANK, dtype=np.float32)
    dinv_rank[:N] = dinv[order]
    dpc = np.zeros((NCORES, P, T), dtype=np.float32)
    for c in range(NCORES):
        rr = (np.arange(T) * 1024)[None, :] + c * 128 + np.arange(P)[:, None]
        dpc[c] = dinv_rank[rr]

    # xTe [XROWS, NROWS]: col g holds x[node].T; row 770 = 1 for real cols
    xTe = np.zeros((XROWS, NROWS), dtype=np.float32)
    xTe[:IN_CH, g_of_node] = np.asarray(x, dtype=np.float32).T
    xTe[IN_CH, g_of_node] = 1.0

    layout = {
        "K": K,
        "chunks": chunks,
        "chunk_meta": chunk_meta,
        "totc": totc_all,
        "off": off,
    }
    return layout, per_core_idxs, dpc, xTe, g_of_node


def dma_gather_raw(nc, out_ap, in_ap, idxs_ap, num_idxs, elem_size, elem_step, queue_num=0):
    """bass dma_gather without the %256 elem-size assert (non-transpose, HBM
    source, multi-packet). Row stride (elem_step * 4B) must be %256 == 0."""
    gp = nc.gpsimd
    stride_bytes = elem_step * mybir.dt.size(in_ap.dtype)
    assert stride_bytes % 256 == 0 and stride_bytes // 256 < 256
    return gp.add_instruction(
        mybir.InstDMAGatherAnt(
            name=nc.get_next_instruction_name(),
            ins=[
                *gp.lower_ap_dma(in_ap, for_custom_bir_dma=True),
                gp.lower_ap(idxs_ap),
                gp.lower_val_access(gp.to_reg(num_idxs)),
            ],
            outs=[gp.lower_ap(out_ap)],
            transpose=False,
            num_idxs=num_idxs,
            elem_size=elem_size,
            stride_bytes_256=stride_bytes // 256,
            gen_mode=0,
            single_packet=False,
            queue_num=queue_num,
            sbuf_tokens_per_rank=0,
            sbuf_free_dim_per_rank=0,
            sbuf_free_dim_pad_per_rank=0,
            sbuf_byte_offset=0,
        )
    )


def build(layout):
    K = layout["K"]
    chunks = layout["chunks"]
    chunk_meta = layout["chunk_meta"]
    totc = layout["totc"]
    off = layout["off"]

    nc = bacc.Bacc("TRN2", num_devices=NCORES, debug=False, num_swdge_queues=4)

    xTe = nc.dram_tensor("xTe", [XROWS, LOCAL], BF16, kind="ExternalInput")
    w1 = nc.dram_tensor("w1", [XCH, NCHUNK_IN * 64], BF16, kind="ExternalInput")
    w2 = nc.dram_tensor("w2", [64, IN_CH], F32R, kind="ExternalInput")
    idxs = nc.dram_tensor("idxs", [P, totc * 8], I16, kind="ExternalInput")
    dinv_in = nc.dram_tensor("dinv", [P, T], F32, kind="ExternalInput")
    out = nc.dram_tensor("out", [LOCAL, IN_CH], F32, kind="ExternalOutput")

    z1loc = nc.dram_tensor("z1loc", [LOCAL, ROWE], F32)
    hloc = nc.dram_tensor("hloc", [LOCAL, ROWE], F32)
    z1g = nc.dram_tensor("z1g", [NROWS, ROWE], F32, addr_space="Shared")
    hg = nc.dram_tensor("hg", [NROWS, ROWE], F32, addr_space="Shared")

    stsizes = [512] * 12 + [128]  # node supertiles (6272 total)

    with tile.TileContext(nc) as tc:
        with (
            tc.tile_pool(name="persist", bufs=1) as pp,
            tc.tile_pool(name="xload", bufs=3) as xp,
            tc.tile_pool(name="gather", bufs=5) as gp_pool,
            tc.tile_pool(name="work", bufs=2) as wp,
            tc.tile_pool(name="outsb", bufs=3) as op_pool,
            tc.tile_pool(name="l1ps", bufs=2, space="PSUM") as l1ps,
            tc.tile_pool(name="trps", bufs=2, space="PSUM") as trps,
            tc.tile_pool(name="outps", bufs=2, space="PSUM") as outps,
        ):
            # ---- persistent loads ----
            w1_sb = pp.tile([XCH, NCHUNK_IN * 64], BF16)
            nc.sync.dma_start(w1_sb[:], w1[:])
            w2_sb = pp.tile([64, IN_CH], F32R)
            nc.sync.dma_start(w2_sb[:], w2[:])
            idxs_sb = pp.tile([P, totc * 8], I16)
            nc.sync.dma_start(idxs_sb[:], idxs[:])
            dinv_sb = pp.tile([P, T], F32)
            nc.sync.dma_start(dinv_sb[:], dinv_in[:])
            ident = pp.tile([P, P], F32)
            make_identity(nc, ident)

            z1r_sb = pp.tile([P, T * 64], F32)  # [z1 | r1+b1] per tile
            slotred = pp.tile([P, T * HID], F32)
            slotred2 = pp.tile([P, T * HID], F32)
            h_sb = pp.tile([P, T * HID], F32)
            tmp_sb = pp.tile([P, T * HID], F32)
            ag2h = pp.tile([P, T * 64], F32)  # [agg2 | h] per tile

            w1v = w1_sb[:].rearrange("p (k c) -> p k c", k=NCHUNK_IN)

            # ---- layer-1 matmul: z1|r1b = xTe_aug @ W1cat ----
            tglob = 0
            for st, stn in enumerate(stsizes):
                xsb = xp.tile([XCH, NCHUNK_IN, 512], BF16, tag="xsb")
                src = xTe.ap().rearrange("(k q) n -> q k n", q=XCH)[
                    :, :, st * 512 : st * 512 + stn
                ]
                nc.sync.dma_start(xsb[:, :, :stn], src)
                for tloc in range(stn // 128):
                    ps = l1ps.tile([P, 64], F32, space="PSUM")
                    for k in range(NCHUNK_IN):
                        nc.tensor.matmul(
                            out=ps[:],
                            lhsT=xsb[:, k, tloc * 128 : (tloc + 1) * 128],
                            rhs=w1v[:, k, :],
                            start=(k == 0),
                            stop=(k == NCHUNK_IN - 1),
                        )
                    nc.vector.tensor_copy(
                        z1r_sb[:, tglob * 64 : (tglob + 1) * 64], ps[:]
                    )
                    tglob += 1

            # ---- store z1 rows, AllGather ----
            z1v = z1r_sb[:].rearrange("p (t d) -> p t d", t=T)
            z1dst = z1loc.ap().rearrange("(t p) c -> p t c", p=P)[:, :, 0:HID]
            nc.sync.dma_start(z1dst, z1v[:, :, 0:HID])
            nc.gpsimd.collective_compute(
                "AllGather",
                mybir.AluOpType.bypass,
                replica_groups=[list(range(NCORES))],
                ins=[z1loc.ap().opt()],
                outs=[z1g.ap().opt()],
            )

            # ---- gather + reduce helper ----
            def gather_layer(table, dest_red, sems, chunk_cb=None):
                gbufs = []
                for ci, (icol, cols_ch, runs) in enumerate(chunk_meta):
                    gbuf = gp_pool.tile([P, MAXCOLS, HID], F32, tag="gbuf")
                    gbufs.append(gbuf)
                    with tc.tile_critical(no_gpsimd_drain=True):
                        dma_gather_raw(
                            nc,
                            gbuf[:, :cols_ch, :],
                            table[BASE:, :],
                            idxs_sb[:, icol * 8 : (icol + cols_ch) * 8],
                            num_idxs=cols_ch * 128,
                            elem_size=HID,
                            elem_step=ROWE,
                            queue_num=ci % 4,
                        ).then_inc(sems[ci], 16)
                for ci, (icol, cols_ch, runs) in enumerate(chunk_meta):
                    gbuf = gbufs[ci]
                    with tc.tile_critical():
                        nc.vector.wait_ge(sems[ci], 16)
                        for (coff, nt, kk, t0) in runs:
                            inv = gbuf[:, coff : coff + nt * kk, :].rearrange(
                                "p (t k) c -> p t c k", k=kk
                            )
                            nc.vector.tensor_reduce(
                                out=dest_red[:, t0 * HID : (t0 + nt) * HID],
                                in_=inv,
                                axis=mybir.AxisListType.X,
                                op=mybir.AluOpType.add,
                            )
                    if chunk_cb is not None:
                        chunk_cb(ci)

            sems1 = [nc.alloc_semaphore(f"g1_{i}") for i in range(len(chunk_meta))]
            gather_layer(z1g, slotred, sems1)

            # ---- h = relu(dinv*(slotred + 2*z1) + r1b) ----
            dinv_b = dinv_sb[:].to_broadcast([P, T, HID])
            sr_v = slotred[:].rearrange("p (t c) -> p t c", t=T)
            tmp_v = tmp_sb[:].rearrange("p (t c) -> p t c", t=T)
            h_v = h_sb[:].rearrange("p (t c) -> p t c", t=T)
            nc.vector.tensor_scalar(
                out=tmp_v, in0=z1v[:, :, 0:HID], scalar1=2.0, scalar2=None,
                op0=mybir.AluOpType.mult,
            )
            nc.vector.tensor_tensor(
                out=tmp_sb[:], in0=tmp_sb[:], in1=slotred[:], op=mybir.AluOpType.add
            )
            nc.vector.tensor_tensor(
                out=tmp_v, in0=tmp_v, in1=dinv_b, op=mybir.AluOpType.mult
            )
            nc.vector.tensor_tensor(
                out=tmp_v, in0=tmp_v, in1=z1v[:, :, HID:64],
                op=mybir.AluOpType.add,
            )
            nc.vector.tensor_scalar(
                out=h_sb[:], in0=tmp_sb[:], scalar1=0.0, scalar2=None,
                op0=mybir.AluOpType.max,
            )

            ag2h_v = ag2h[:].rearrange("p (t d) -> p t d", t=T)
            nc.vector.tensor_copy(ag2h_v[:, :, HID:64], h_v)

            # ---- store h rows, AllGather ----
            hdst = hloc.ap().rearrange("(t p) c -> p t c", p=P)[:, :, 0:HID]
            nc.sync.dma_start(hdst, h_v)
            nc.gpsimd.collective_compute(
                "AllGather",
                mybir.AluOpType.bypass,
                replica_groups=[list(range(NCORES))],
                ins=[hloc.ap().opt()],
                outs=[hg.ap().opt()],
            )

            # ---- L2: per-chunk assembly + output pipeline ----
            def l2_chunk(ci):
                a, b = chunks[ci]
                nt = b - a
                sl = slice(a * HID, b * HID)
                tv = tmp_sb[:, sl].rearrange("p (t c) -> p t c", t=nt)
                nc.vector.tensor_scalar(
                    out=tmp_sb[:, sl], in0=h_sb[:, sl], scalar1=2.0, scalar2=None,
                    op0=mybir.AluOpType.mult,
                )
                nc.vector.tensor_tensor(
                    out=tmp_sb[:, sl], in0=tmp_sb[:, sl], in1=slotred2[:, sl],
                    op=mybir.AluOpType.add,
                )
                nc.vector.tensor_tensor(
                    out=ag2h_v[:, a:b, 0:HID], in0=tv,
                    in1=dinv_sb[:, a:b].to_broadcast([P, nt, HID]),
                    op=mybir.AluOpType.mult,
                )
                for t in range(a, b):
                    tp = trps.tile([64, P], F32, space="PSUM")
                    nc.tensor.transpose(
                        out=tp[:], in_=ag2h[:, t * 64 : (t + 1) * 64],
                        identity=ident[:],
                    )
                    catT = wp.tile([64, P], F32R, tag="catT")
                    nc.vector.tensor_copy(catT[:], tp[:])
                    pso = outps.tile([P, IN_CH], F32, space="PSUM")
                    nc.tensor.matmul(
                        out=pso[:, 0:512], lhsT=catT[:], rhs=w2_sb[:, 0:512],
                        start=True, stop=True,
                    )
                    nc.tensor.matmul(
                        out=pso[:, 512:IN_CH], lhsT=catT[:], rhs=w2_sb[:, 512:IN_CH],
                        start=True, stop=True,
                    )
                    osb = op_pool.tile([P, IN_CH], F32, tag="osb")
                    nc.vector.tensor_copy(osb[:], pso[:])
                    nc.sync.dma_start(out[t * P : (t + 1) * P, :], osb[:])

            sems2 = [nc.alloc_semaphore(f"g2_{i}") for i in range(len(chunk_meta))]
            gather_layer(hg, slotred2, sems2, chunk_cb=l2_chunk)

    nc.compile()
    return nc


_CACHE = {}


def kernel(x, edge_index, W_out1, b_out1, W_root1, W_out2, b_out2, W_root2):
    x = np.asarray(x, dtype=np.float32)
    layout, per_core_idxs, dpc, xTe, g_of_node = _host_prep(x, edge_index)

    # weight tensors
    w1cat = np.zeros((XROWS, 64), dtype=np.float32)
    w1cat[:IN_CH, :HID] = np.asarray(W_out1, dtype=np.float32).T
    w1cat[:IN_CH, HID:] = np.asarray(W_root1, dtype=np.float32).T
    w1cat[IN_CH, HID:] = np.asarray(b_out1, dtype=np.float32)
    # rearrange to [XCH, NCHUNK_IN*64]: [p, k*64+c] = w1cat[k*XCH+p, c]
    import ml_dtypes

    w1_dev = np.ascontiguousarray(
        w1cat.reshape(NCHUNK_IN, XCH, 64).transpose(1, 0, 2).reshape(XCH, -1)
    ).astype(ml_dtypes.bfloat16)
    w2_dev = np.concatenate(
        [np.asarray(W_out2, dtype=np.float32).T, np.asarray(W_root2, dtype=np.float32).T],
        axis=0,
    )  # [64, 770]

    key = (tuple(int(k) for k in layout["K"]), tuple(layout["chunks"]))
    if key not in _CACHE:
        _CACHE[key] = build(layout)
    nc = _CACHE[key]

    in_maps = []
    for c in range(NCORES):
        in_maps.append(
            {
                "xTe": np.ascontiguousarray(xTe[:, c * LOCAL : (c + 1) * LOCAL]).astype(ml_dtypes.bfloat16),
                "w1": w1_dev,
                "w2": np.ascontiguousarray(w2_dev),
                "idxs": per_core_idxs[c],
                "dinv": dpc[c],
            }
        )

    trace = os.environ.get("GCN_TRACE", "0") == "1"
    res = run_bass_kernel_spmd(
        nc, in_maps, core_ids=list(range(NCORES)), trace=trace
    )
    if trace and res.exec_time_ns is not None:
        print(f"HW exec time: {res.exec_time_ns} ns")
        kernel.last_exec_time_ns = res.exec_time_ns

    out = np.empty((N, IN_CH), dtype=np.float32)
    # table row g -> node
    node_of_g = np.full(NROWS, -1, dtype=np.int64)
    node_of_g[g_of_node] = np.arange(N)
    for c in range(NCORES):
        gs = np.arange(c * LOCAL, (c + 1) * LOCAL)
        nodes = node_of_g[gs]
        m = nodes >= 0
        out[nodes[m]] = res.results[c]["out"][m]
    out += np.asarray(b_out2, dtype=np.float32)[None, :]
    return out


# revision 7
# speedup vs baseline: 1.2878x; 1.0173x over previous
"""ClusterGCN 2-layer kernel for 8 Trainium2 NeuronCores.

Strategy:
 - Exploit linearity: project x (770ch) down to 32ch FIRST (z1 = x @ W_out1.T),
   then message-pass on 32-dim vectors (24x less gather traffic).
 - Edge weight = deg_inv[dest] (uniform per destination) => aggregate raw
   neighbor sums, scale once per destination.
 - Nodes degree-sorted and dealt across 8 cores; per-destination padded slot
   lists (gather indices) shared by both layers.
 - Device: z1|r1 via PE matmul, AllGather z1 (row-major table, 256B stride),
   one dma_gather per chunk of destination tiles (int16 signed wrapped
   indices with +32768-row base), strided middle-axis tensor_reduce,
   elementwise assembly of h, AllGather h, same gathers again, PE transpose,
   final f32r matmul to [128, 770] output tiles.
"""

import os
import sys
import types

import numpy as np

# ---------------------------------------------------------------------------
# environment shims (axon NTFF hook + no artifact bucket)
# ---------------------------------------------------------------------------
if "antenv.axon_hooks" not in sys.modules:
    _mod = types.ModuleType("antenv.axon_hooks")
    _hook_store = [None]
    _mod.set_axon_ntff_profile_hook = lambda h: _hook_store.__setitem__(0, h)
    _mod.get_axon_ntff_profile_hook = lambda: _hook_store[0]
    try:
        import antenv

        antenv.axon_hooks = _mod
        sys.modules["antenv.axon_hooks"] = _mod
        from trn_agent_boot.trn_boot import _ntff_profile_via_ctypes

        _mod.set_axon_ntff_profile_hook(
            _ntff_profile_via_ctypes("/opt/axon/libaxon_pjrt.so")
        )
    except Exception:
        pass

import concourse.bacc as bacc
import concourse.bass as bass
import concourse.bass_utils as bass_utils
import concourse.mybir as mybir
import concourse.tile as tile
from concourse.bass_utils import run_bass_kernel_spmd
from concourse.masks import make_identity

bass_utils.upload_artifacts = lambda tmpdir: tmpdir

# ---------------------------------------------------------------------------
# problem constants (hardcoded per the harness contract)
# ---------------------------------------------------------------------------
N = 50000
E = 400000
IN_CH = 770
HID = 32
DIAG_LAMBDA = 1.0
NCORES = 8
P = 128
T = 49  # destination tiles per core
LOCAL = T * P  # 6272 rows per core block
NRANK = T * 1024  # 50176 ranks total
NROWS = NCORES * LOCAL  # 50176 gather-table rows
BASE = 32768  # gather base-row offset (signed int16 wrap)
ROWE = 64  # table row stride in f32 elems (256B)
XCH = 112  # input-channel chunk (7 chunks x 112 = 784)
NCHUNK_IN = 7
XROWS = NCHUNK_IN * XCH  # 784 = 770 + ones row + 13 zero rows
MAXCOLS = 56  # max gather columns per dma_gather chunk

F32 = mybir.dt.float32
BF16 = mybir.dt.bfloat16
F32R = mybir.dt.float32r
I16 = mybir.dt.int16


def _host_prep(x, edge_index):
    """Degree stats, node permutation, gather slot tables, xTe."""
    row = np.asarray(edge_index[0], dtype=np.int64)
    col = np.asarray(edge_index[1], dtype=np.int64)
    ns = row != col
    r_, c_ = row[ns], col[ns]
    indeg = np.bincount(c_, minlength=N)
    deg = (indeg + 1).astype(np.float64)
    dinv = (1.0 / deg).astype(np.float32)

    order = np.argsort(-indeg, kind="stable")  # rank -> node
    rank_of = np.empty(N, dtype=np.int64)
    rank_of[order] = np.arange(N)

    r_all = np.arange(NRANK)
    core_of_rank = (r_all % 1024) // 128
    l_of_rank = (r_all // 1024) * 128 + (r_all % 128)
    g_of_rank = core_of_rank * LOCAL + l_of_rank
    g_of_node = g_of_rank[rank_of]  # node -> table row

    indeg_rank = np.zeros(NRANK, dtype=np.int64)
    indeg_rank[:N] = indeg[order]
    K = np.maximum(indeg_rank[np.arange(T) * 1024], 1).astype(np.int64)
    off = np.concatenate([[0], np.cumsum(K)])
    totk = int(off[-1])

    # chunks of consecutive tiles, each <= MAXCOLS gather columns (+1 pad col)
    chunks = []
    t0 = 0
    acc = 0
    for t in range(T):
        if acc and acc + K[t] > MAXCOLS - 1:
            chunks.append((t0, t))
            t0 = t
            acc = 0
        acc += int(K[t])
    chunks.append((t0, T))

    # slot table [8, totk, 128] of table-row g values, init to pad rows
    padg = g_of_rank[N:NRANK]  # 176 all-zero rows (cores 6,7 tails)
    init = padg[np.arange(8 * totk * 128) % len(padg)]
    slot_g = init.reshape(8, totk, 128)

    dest_rank = rank_of[c_]
    sidx = np.argsort(dest_rank, kind="stable")
    dr = dest_rank[sidx]
    src_g = g_of_node[r_[sidx]]
    cnt = np.bincount(dr, minlength=NRANK)
    cum = np.concatenate([[0], np.cumsum(cnt)])
    within = np.arange(len(dr)) - cum[dr]
    t_d = dr // 1024
    c_d = (dr % 1024) // 128
    p_d = dr % 128
    colg = off[t_d] + within
    slot_g[c_d, colg, p_d] = src_g

    # final per-core index arrays with chunk pad columns appended
    wrapped = (slot_g - BASE).astype(np.int16)
    padcol = (padg[np.arange(128) % len(padg)] - BASE).astype(np.int16)  # >0
    per_core_idxs = []
    chunk_meta = []  # (idx_col_off, cols_ch, runs, gbuf_tile_offs)
    for c in range(NCORES):
        parts = []
        icol = 0
        for (a, b) in chunks:
            cols_ch = int(off[b] - off[a]) + 1
            parts.append(wrapped[c, off[a] : off[b], :])
            parts.append(padcol[None, :])
            if c == 0:
                # reduce runs: consecutive tiles with equal K
                runs = []
                t = a
                while t < b:
                    t2 = t
                    while t2 < b and K[t2] == K[t]:
                        t2 += 1
                    runs.append(
                        (int(off[t] - off[a]), t2 - t, int(K[t]), t)
                    )  # (col_off_in_chunk, ntiles, K, tile0)
                    t = t2
                chunk_meta.append((icol, cols_ch, runs))
            icol += cols_ch
        allcols = np.concatenate(parts, axis=0)  # [TOTC, 128]
        totc = allcols.shape[0]
        flat = allcols.reshape(-1)  # position j = colc*128 + p
        a16 = np.zeros((16, totc * 8), dtype=np.int16)
        j = np.arange(totc * 128)
        a16[j % 16, j // 16] = flat
        per_core_idxs.append(np.tile(a16, (8, 1)))
    totc_all = per_core_idxs[0].shape[1] // 8

    # per-core dinv [128, T]
    dinv_rank = np.zeros(NRANK, dtype=np.float32)
    dinv_rank[:N] = dinv[order]
    dpc = np.zeros((NCORES, P, T), dtype=np.float32)
    for c in range(NCORES):
        rr = (np.arange(T) * 1024)[None, :] + c * 128 + np.arange(P)[:, None]
        dpc[c] = dinv_rank[rr]

    # xTe [XROWS, NROWS]: col g holds x[node].T; row 770 = 1 for real cols
    xTe = np.zeros((XROWS, NROWS), dtype=np.float32)
    xTe[:IN_CH, g_of_node] = np.asarray(x, dtype=np.float32).T
    xTe[IN_CH, g_of_node] = 1.0

    layout = {
        "K": K,
        "chunks": chunks,
        "chunk_meta": chunk_meta,
        "totc": totc_all,
        "off": off,
    }
    return layout, per_core_idxs, dpc, xTe, g_of_node


def dma_gather_raw(nc, out_ap, in_ap, idxs_ap, num_idxs, elem_size, elem_step, queue_num=0):
    """bass dma_gather without the %256 elem-size assert (non-transpose, HBM
    source, multi-packet). Row stride (elem_step * 4B) must be %256 == 0."""
    gp = nc.gpsimd
    stride_bytes = elem_step * mybir.dt.size(in_ap.dtype)
    assert stride_bytes % 256 == 0 and stride_bytes // 256 < 256
    return gp.add_instruction(
        mybir.InstDMAGatherAnt(
            name=nc.get_next_instruction_name(),
            ins=[
                *gp.lower_ap_dma(in_ap, for_custom_bir_dma=True),
                gp.lower_ap(idxs_ap),
                gp.lower_val_access(gp.to_reg(num_idxs)),
            ],
            outs=[gp.lower_ap(out_ap)],
            transpose=False,
            num_idxs=num_idxs,
            elem_size=elem_size,
            stride_bytes_256=stride_bytes // 256,
            gen_mode=0,
            single_packet=False,
            queue_num=queue_num,
            sbuf_tokens_per_rank=0,
            sbuf_free_dim_per_rank=0,
            sbuf_free_dim_pad_per_rank=0,
            sbuf_byte_offset=0,
        )
    )


def build(layout):
    K = layout["K"]
    chunks = layout["chunks"]
    chunk_meta = layout["chunk_meta"]
    totc = layout["totc"]
    off = layout["off"]

    nc = bacc.Bacc("TRN2", num_devices=NCORES, debug=False, num_swdge_queues=4)

    xTe = nc.dram_tensor("xTe", [XROWS, LOCAL], BF16, kind="ExternalInput")
    w1 = nc.dram_tensor("w1", [XCH, NCHUNK_IN * 64], BF16, kind="ExternalInput")
    w2 = nc.dram_tensor("w2", [64, IN_CH], F32R, kind="ExternalInput")
    idxs = nc.dram_tensor("idxs", [P, totc * 8], I16, kind="ExternalInput")
    dinv_in = nc.dram_tensor("dinv", [P, T], F32, kind="ExternalInput")
    out = nc.dram_tensor("out", [LOCAL, IN_CH], F32, kind="ExternalOutput")

    z1loc = nc.dram_tensor("z1loc", [LOCAL, ROWE], F32)
    hloc = nc.dram_tensor("hloc", [LOCAL, ROWE], F32)
    z1g = nc.dram_tensor("z1g", [NROWS, ROWE], F32, addr_space="Shared")
    hg = nc.dram_tensor("hg", [NROWS, ROWE], F32, addr_space="Shared")

    stsizes = [512] * 12 + [128]  # node supertiles (6272 total)

    with tile.TileContext(nc) as tc:
        with (
            tc.tile_pool(name="persist", bufs=1) as pp,
            tc.tile_pool(name="xload", bufs=3) as xp,
            tc.tile_pool(name="gather", bufs=9) as gp_pool,
            tc.tile_pool(name="work", bufs=2) as wp,
            tc.tile_pool(name="outsb", bufs=3) as op_pool,
            tc.tile_pool(name="l1ps", bufs=2, space="PSUM") as l1ps,
            tc.tile_pool(name="trps", bufs=2, space="PSUM") as trps,
            tc.tile_pool(name="outps", bufs=2, space="PSUM") as outps,
        ):
            # ---- persistent loads ----
            w1_sb = pp.tile([XCH, NCHUNK_IN * 64], BF16)
            nc.sync.dma_start(w1_sb[:], w1[:])
            w2_sb = pp.tile([64, IN_CH], F32R)
            nc.sync.dma_start(w2_sb[:], w2[:])
            idxs_sb = pp.tile([P, totc * 8], I16)
            nc.sync.dma_start(idxs_sb[:], idxs[:])
            dinv_sb = pp.tile([P, T], F32)
            nc.sync.dma_start(dinv_sb[:], dinv_in[:])
            ident = pp.tile([P, P], F32)
            make_identity(nc, ident)

            z1r_sb = pp.tile([P, T * 64], F32)  # [z1 | r1+b1] per tile
            slotred = pp.tile([P, T * HID], F32)
            slotred2 = pp.tile([P, T * HID], F32)
            h_sb = pp.tile([P, T * HID], F32)
            tmp_sb = pp.tile([P, T * HID], F32)
            ag2h = pp.tile([P, T * 64], F32)  # [agg2 | h] per tile

            w1v = w1_sb[:].rearrange("p (k c) -> p k c", k=NCHUNK_IN)

            # ---- layer-1 matmul: z1|r1b = xTe_aug @ W1cat ----
            tglob = 0
            for st, stn in enumerate(stsizes):
                xsb = xp.tile([XCH, NCHUNK_IN, 512], BF16, tag="xsb")
                src = xTe.ap().rearrange("(k q) n -> q k n", q=XCH)[
                    :, :, st * 512 : st * 512 + stn
                ]
                nc.sync.dma_start(xsb[:, :, :stn], src)
                for tloc in range(stn // 128):
                    ps = l1ps.tile([P, 64], F32, space="PSUM")
                    for k in range(NCHUNK_IN):
                        nc.tensor.matmul(
                            out=ps[:],
                            lhsT=xsb[:, k, tloc * 128 : (tloc + 1) * 128],
                            rhs=w1v[:, k, :],
                            start=(k == 0),
                            stop=(k == NCHUNK_IN - 1),
                        )
                    nc.vector.tensor_copy(
                        z1r_sb[:, tglob * 64 : (tglob + 1) * 64], ps[:]
                    )
                    tglob += 1

            # ---- store z1 rows, AllGather ----
            z1v = z1r_sb[:].rearrange("p (t d) -> p t d", t=T)
            z1dst = z1loc.ap().rearrange("(t p) c -> p t c", p=P)[:, :, 0:HID]
            nc.sync.dma_start(z1dst, z1v[:, :, 0:HID])
            nc.gpsimd.collective_compute(
                "AllGather",
                mybir.AluOpType.bypass,
                replica_groups=[list(range(NCORES))],
                ins=[z1loc.ap().opt()],
                outs=[z1g.ap().opt()],
            )

            # ---- gather + reduce helper ----
            def gather_layer(table, dest_red, sems, chunk_cb=None):
                gbufs = []
                for ci, (icol, cols_ch, runs) in enumerate(chunk_meta):
                    gbuf = gp_pool.tile([P, MAXCOLS, HID], F32, tag="gbuf")
                    gbufs.append(gbuf)
                    with tc.tile_critical(no_gpsimd_drain=True):
                        dma_gather_raw(
                            nc,
                            gbuf[:, :cols_ch, :],
                            table[BASE:, :],
                            idxs_sb[:, icol * 8 : (icol + cols_ch) * 8],
                            num_idxs=cols_ch * 128,
                            elem_size=HID,
                            elem_step=ROWE,
                            queue_num=ci % 4,
                        ).then_inc(sems[ci], 16)
                for ci, (icol, cols_ch, runs) in enumerate(chunk_meta):
                    gbuf = gbufs[ci]
                    with tc.tile_critical():
                        nc.vector.wait_ge(sems[ci], 16)
                        for (coff, nt, kk, t0) in runs:
                            inv = gbuf[:, coff : coff + nt * kk, :].rearrange(
                                "p (t k) c -> p t c k", k=kk
                            )
                            nc.vector.tensor_reduce(
                                out=dest_red[:, t0 * HID : (t0 + nt) * HID],
                                in_=inv,
                                axis=mybir.AxisListType.X,
                                op=mybir.AluOpType.add,
                            )
                    if chunk_cb is not None:
                        chunk_cb(ci)

            sems1 = [nc.alloc_semaphore(f"g1_{i}") for i in range(len(chunk_meta))]
            gather_layer(z1g, slotred, sems1)

            # ---- h = relu(dinv*(slotred + 2*z1) + r1b) ----
            dinv_b = dinv_sb[:].to_broadcast([P, T, HID])
            sr_v = slotred[:].rearrange("p (t c) -> p t c", t=T)
            tmp_v = tmp_sb[:].rearrange("p (t c) -> p t c", t=T)
            h_v = h_sb[:].rearrange("p (t c) -> p t c", t=T)
            nc.vector.tensor_scalar(
                out=tmp_v, in0=z1v[:, :, 0:HID], scalar1=2.0, scalar2=None,
                op0=mybir.AluOpType.mult,
            )
            nc.vector.tensor_tensor(
                out=tmp_sb[:], in0=tmp_sb[:], in1=slotred[:], op=mybir.AluOpType.add
            )
            nc.vector.tensor_tensor(
                out=tmp_v, in0=tmp_v, in1=dinv_b, op=mybir.AluOpType.mult
            )
            nc.vector.tensor_tensor(
                out=tmp_v, in0=tmp_v, in1=z1v[:, :, HID:64],
                op=mybir.AluOpType.add,
            )
            nc.vector.tensor_scalar(
                out=h_sb[:], in0=tmp_sb[:], scalar1=0.0, scalar2=None,
                op0=mybir.AluOpType.max,
            )

            ag2h_v = ag2h[:].rearrange("p (t d) -> p t d", t=T)
            nc.vector.tensor_copy(ag2h_v[:, :, HID:64], h_v)

            # ---- store h rows, AllGather ----
            hdst = hloc.ap().rearrange("(t p) c -> p t c", p=P)[:, :, 0:HID]
            nc.sync.dma_start(hdst, h_v)
            nc.gpsimd.collective_compute(
                "AllGather",
                mybir.AluOpType.bypass,
                replica_groups=[list(range(NCORES))],
                ins=[hloc.ap().opt()],
                outs=[hg.ap().opt()],
            )

            # ---- L2: per-chunk assembly + output pipeline ----
            def l2_chunk(ci):
                a, b = chunks[ci]
                nt = b - a
                sl = slice(a * HID, b * HID)
                tv = tmp_sb[:, sl].rearrange("p (t c) -> p t c", t=nt)
                nc.vector.tensor_scalar(
                    out=tmp_sb[:, sl], in0=h_sb[:, sl], scalar1=2.0, scalar2=None,
                    op0=mybir.AluOpType.mult,
                )
                nc.vector.tensor_tensor(
                    out=tmp_sb[:, sl], in0=tmp_sb[:, sl], in1=slotred2[:, sl],
                    op=mybir.AluOpType.add,
                )
                nc.vector.tensor_tensor(
                    out=ag2h_v[:, a:b, 0:HID], in0=tv,
                    in1=dinv_sb[:, a:b].to_broadcast([P, nt, HID]),
                    op=mybir.AluOpType.mult,
                )
                for t in range(a, b):
                    tp = trps.tile([64, P], F32, space="PSUM")
                    nc.tensor.transpose(
                        out=tp[:], in_=ag2h[:, t * 64 : (t + 1) * 64],
                        identity=ident[:],
                    )
                    catT = wp.tile([64, P], F32R, tag="catT")
                    nc.vector.tensor_copy(catT[:], tp[:])
                    pso = outps.tile([P, IN_CH], F32, space="PSUM")
                    nc.tensor.matmul(
                        out=pso[:, 0:512], lhsT=catT[:], rhs=w2_sb[:, 0:512],
                        start=True, stop=True,
                    )
                    nc.tensor.matmul(
                        out=pso[:, 512:IN_CH], lhsT=catT[:], rhs=w2_sb[:, 512:IN_CH],
                        start=True, stop=True,
                    )
                    osb = op_pool.tile([P, IN_CH], F32, tag="osb")
                    nc.vector.tensor_copy(osb[:], pso[:])
                    nc.sync.dma_start(out[t * P : (t + 1) * P, :], osb[:])

            sems2 = [nc.alloc_semaphore(f"g2_{i}") for i in range(len(chunk_meta))]
            gather_layer(hg, slotred2, sems2, chunk_cb=l2_chunk)

    nc.compile()
    return nc


_CACHE = {}


def kernel(x, edge_index, W_out1, b_out1, W_root1, W_out2, b_out2, W_root2):
    x = np.asarray(x, dtype=np.float32)
    layout, per_core_idxs, dpc, xTe, g_of_node = _host_prep(x, edge_index)

    # weight tensors
    w1cat = np.zeros((XROWS, 64), dtype=np.float32)
    w1cat[:IN_CH, :HID] = np.asarray(W_out1, dtype=np.float32).T
    w1cat[:IN_CH, HID:] = np.asarray(W_root1, dtype=np.float32).T
    w1cat[IN_CH, HID:] = np.asarray(b_out1, dtype=np.float32)
    # rearrange to [XCH, NCHUNK_IN*64]: [p, k*64+c] = w1cat[k*XCH+p, c]
    import ml_dtypes

    w1_dev = np.ascontiguousarray(
        w1cat.reshape(NCHUNK_IN, XCH, 64).transpose(1, 0, 2).reshape(XCH, -1)
    ).astype(ml_dtypes.bfloat16)
    w2_dev = np.concatenate(
        [np.asarray(W_out2, dtype=np.float32).T, np.asarray(W_root2, dtype=np.float32).T],
        axis=0,
    )  # [64, 770]

    key = (tuple(int(k) for k in layout["K"]), tuple(layout["chunks"]))
    if key not in _CACHE:
        _CACHE[key] = build(layout)
    nc = _CACHE[key]

    in_maps = []
    for c in range(NCORES):
        in_maps.append(
            {
                "xTe": np.ascontiguousarray(xTe[:, c * LOCAL : (c + 1) * LOCAL]).astype(ml_dtypes.bfloat16),
                "w1": w1_dev,
                "w2": np.ascontiguousarray(w2_dev),
                "idxs": per_core_idxs[c],
                "dinv": dpc[c],
            }
        )

    trace = os.environ.get("GCN_TRACE", "0") == "1"
    res = run_bass_kernel_spmd(
        nc, in_maps, core_ids=list(range(NCORES)), trace=trace
    )
    if trace and res.exec_time_ns is not None:
        print(f"HW exec time: {res.exec_time_ns} ns")
        kernel.last_exec_time_ns = res.exec_time_ns

    out = np.empty((N, IN_CH), dtype=np.float32)
    # table row g -> node
    node_of_g = np.full(NROWS, -1, dtype=np.int64)
    node_of_g[g_of_node] = np.arange(N)
    for c in range(NCORES):
        gs = np.arange(c * LOCAL, (c + 1) * LOCAL)
        nodes = node_of_g[gs]
        m = nodes >= 0
        out[nodes[m]] = res.results[c]["out"][m]
    out += np.asarray(b_out2, dtype=np.float32)[None, :]
    return out


# revision 8
# speedup vs baseline: 1.3064x; 1.0145x over previous
"""ClusterGCN 2-layer kernel for 8 Trainium2 NeuronCores.

Strategy:
 - Exploit linearity: project x (770ch) down to 32ch FIRST (z1 = x @ W_out1.T),
   then message-pass on 32-dim vectors (24x less gather traffic).
 - Edge weight = deg_inv[dest] (uniform per destination) => aggregate raw
   neighbor sums, scale once per destination.
 - Nodes degree-sorted and dealt across 8 cores; per-destination padded slot
   lists (gather indices) shared by both layers.
 - Device: z1|r1 via PE matmul, AllGather z1 (row-major table, 256B stride),
   one dma_gather per chunk of destination tiles (int16 signed wrapped
   indices with +32768-row base), strided middle-axis tensor_reduce,
   elementwise assembly of h, AllGather h, same gathers again, PE transpose,
   final f32r matmul to [128, 770] output tiles.
"""

import os
import sys
import types

import numpy as np

# ---------------------------------------------------------------------------
# environment shims (axon NTFF hook + no artifact bucket)
# ---------------------------------------------------------------------------
if "antenv.axon_hooks" not in sys.modules:
    _mod = types.ModuleType("antenv.axon_hooks")
    _hook_store = [None]
    _mod.set_axon_ntff_profile_hook = lambda h: _hook_store.__setitem__(0, h)
    _mod.get_axon_ntff_profile_hook = lambda: _hook_store[0]
    try:
        import antenv

        antenv.axon_hooks = _mod
        sys.modules["antenv.axon_hooks"] = _mod
        from trn_agent_boot.trn_boot import _ntff_profile_via_ctypes

        _mod.set_axon_ntff_profile_hook(
            _ntff_profile_via_ctypes("/opt/axon/libaxon_pjrt.so")
        )
    except Exception:
        pass

import concourse.bacc as bacc
import concourse.bass as bass
import concourse.bass_utils as bass_utils
import concourse.mybir as mybir
import concourse.tile as tile
from concourse.bass_utils import run_bass_kernel_spmd
from concourse.masks import make_identity

bass_utils.upload_artifacts = lambda tmpdir: tmpdir

# ---------------------------------------------------------------------------
# problem constants (hardcoded per the harness contract)
# ---------------------------------------------------------------------------
N = 50000
E = 400000
IN_CH = 770
HID = 32
DIAG_LAMBDA = 1.0
NCORES = 8
P = 128
T = 49  # destination tiles per core
LOCAL = T * P  # 6272 rows per core block
NRANK = T * 1024  # 50176 ranks total
NROWS = NCORES * LOCAL  # 50176 gather-table rows
BASE = 32768  # gather base-row offset (signed int16 wrap)
ROWE = 64  # table row stride in f32 elems (256B)
XCH = 112  # input-channel chunk (7 chunks x 112 = 784)
NCHUNK_IN = 7
XROWS = NCHUNK_IN * XCH  # 784 = 770 + ones row + 13 zero rows
MAXCOLS = 56  # max gather columns per dma_gather chunk

F32 = mybir.dt.float32
BF16 = mybir.dt.bfloat16
F32R = mybir.dt.float32r
I16 = mybir.dt.int16


def _host_prep(x, edge_index):
    """Degree stats, node permutation, gather slot tables, xTe."""
    row = np.asarray(edge_index[0], dtype=np.int64)
    col = np.asarray(edge_index[1], dtype=np.int64)
    ns = row != col
    r_, c_ = row[ns], col[ns]
    indeg = np.bincount(c_, minlength=N)
    deg = (indeg + 1).astype(np.float64)
    dinv = (1.0 / deg).astype(np.float32)

    order = np.argsort(-indeg, kind="stable")  # rank -> node
    rank_of = np.empty(N, dtype=np.int64)
    rank_of[order] = np.arange(N)

    r_all = np.arange(NRANK)
    core_of_rank = (r_all % 1024) // 128
    l_of_rank = (r_all // 1024) * 128 + (r_all % 128)
    g_of_rank = core_of_rank * LOCAL + l_of_rank
    g_of_node = g_of_rank[rank_of]  # node -> table row

    indeg_rank = np.zeros(NRANK, dtype=np.int64)
    indeg_rank[:N] = indeg[order]
    K = np.maximum(indeg_rank[np.arange(T) * 1024], 1).astype(np.int64)
    off = np.concatenate([[0], np.cumsum(K)])
    totk = int(off[-1])

    # chunks of consecutive tiles, each <= MAXCOLS gather columns (+1 pad col)
    chunks = []
    t0 = 0
    acc = 0
    for t in range(T):
        if acc and acc + K[t] > MAXCOLS - 1:
            chunks.append((t0, t))
            t0 = t
            acc = 0
        acc += int(K[t])
    chunks.append((t0, T))

    # slot table [8, totk, 128] of table-row g values, init to pad rows
    padg = g_of_rank[N:NRANK]  # 176 all-zero rows (cores 6,7 tails)
    init = padg[np.arange(8 * totk * 128) % len(padg)]
    slot_g = init.reshape(8, totk, 128)

    dest_rank = rank_of[c_]
    sidx = np.argsort(dest_rank, kind="stable")
    dr = dest_rank[sidx]
    src_g = g_of_node[r_[sidx]]
    cnt = np.bincount(dr, minlength=NRANK)
    cum = np.concatenate([[0], np.cumsum(cnt)])
    within = np.arange(len(dr)) - cum[dr]
    t_d = dr // 1024
    c_d = (dr % 1024) // 128
    p_d = dr % 128
    colg = off[t_d] + within
    slot_g[c_d, colg, p_d] = src_g

    # final per-core index arrays with chunk pad columns appended
    wrapped = (slot_g - BASE).astype(np.int16)
    padcol = (padg[np.arange(128) % len(padg)] - BASE).astype(np.int16)  # >0
    per_core_idxs = []
    chunk_meta = []  # (idx_col_off, cols_ch, runs, gbuf_tile_offs)
    for c in range(NCORES):
        parts = []
        icol = 0
        for (a, b) in chunks:
            cols_ch = int(off[b] - off[a]) + 1
            parts.append(wrapped[c, off[a] : off[b], :])
            parts.append(padcol[None, :])
            if c == 0:
                # reduce runs: consecutive tiles with equal K
                runs = []
                t = a
                while t < b:
                    t2 = t
                    while t2 < b and K[t2] == K[t]:
                        t2 += 1
                    runs.append(
                        (int(off[t] - off[a]), t2 - t, int(K[t]), t)
                    )  # (col_off_in_chunk, ntiles, K, tile0)
                    t = t2
                chunk_meta.append((icol, cols_ch, runs))
            icol += cols_ch
        allcols = np.concatenate(parts, axis=0)  # [TOTC, 128]
        totc = allcols.shape[0]
        flat = allcols.reshape(-1)  # position j = colc*128 + p
        a16 = np.zeros((16, totc * 8), dtype=np.int16)
        j = np.arange(totc * 128)
        a16[j % 16, j // 16] = flat
        per_core_idxs.append(np.tile(a16, (8, 1)))
    totc_all = per_core_idxs[0].shape[1] // 8

    # per-core dinv [128, T]
    dinv_rank = np.zeros(NRANK, dtype=np.float32)
    dinv_rank[:N] = dinv[order]
    dpc = np.zeros((NCORES, P, T), dtype=np.float32)
    for c in range(NCORES):
        rr = (np.arange(T) * 1024)[None, :] + c * 128 + np.arange(P)[:, None]
        dpc[c] = dinv_rank[rr]

    # xTe [XROWS, NROWS]: col g holds x[node].T; row 770 = 1 for real cols
    xTe = np.zeros((XROWS, NROWS), dtype=np.float32)
    xTe[:IN_CH, g_of_node] = np.asarray(x, dtype=np.float32).T
    xTe[IN_CH, g_of_node] = 1.0

    layout = {
        "K": K,
        "chunks": chunks,
        "chunk_meta": chunk_meta,
        "totc": totc_all,
        "off": off,
    }
    return layout, per_core_idxs, dpc, xTe, g_of_node


def dma_gather_raw(nc, out_ap, in_ap, idxs_ap, num_idxs, elem_size, elem_step, queue_num=0):
    """bass dma_gather without the %256 elem-size assert (non-transpose, HBM
    source, multi-packet). Row stride (elem_step * 4B) must be %256 == 0."""
    gp = nc.gpsimd
    stride_bytes = elem_step * mybir.dt.size(in_ap.dtype)
    assert stride_bytes % 256 == 0 and stride_bytes // 256 < 256
    return gp.add_instruction(
        mybir.InstDMAGatherAnt(
            name=nc.get_next_instruction_name(),
            ins=[
                *gp.lower_ap_dma(in_ap, for_custom_bir_dma=True),
                gp.lower_ap(idxs_ap),
                gp.lower_val_access(gp.to_reg(num_idxs)),
            ],
            outs=[gp.lower_ap(out_ap)],
            transpose=False,
            num_idxs=num_idxs,
            elem_size=elem_size,
            stride_bytes_256=stride_bytes // 256,
            gen_mode=0,
            single_packet=False,
            queue_num=queue_num,
            sbuf_tokens_per_rank=0,
            sbuf_free_dim_per_rank=0,
            sbuf_free_dim_pad_per_rank=0,
            sbuf_byte_offset=0,
        )
    )


def build(layout):
    K = layout["K"]
    chunks = layout["chunks"]
    chunk_meta = layout["chunk_meta"]
    totc = layout["totc"]
    off = layout["off"]

    nc = bacc.Bacc("TRN2", num_devices=NCORES, debug=False, num_swdge_queues=4)

    xTe = nc.dram_tensor("xTe", [XROWS, LOCAL], BF16, kind="ExternalInput")
    w1 = nc.dram_tensor("w1", [XCH, NCHUNK_IN * 64], BF16, kind="ExternalInput")
    w2 = nc.dram_tensor("w2", [64, IN_CH], F32R, kind="ExternalInput")
    idxs = nc.dram_tensor("idxs", [P, totc * 8], I16, kind="ExternalInput")
    dinv_in = nc.dram_tensor("dinv", [P, T], F32, kind="ExternalInput")
    out = nc.dram_tensor("out", [LOCAL, IN_CH], F32, kind="ExternalOutput")

    z1loc = nc.dram_tensor("z1loc", [LOCAL, ROWE], F32)
    hloc = nc.dram_tensor("hloc", [LOCAL, ROWE], F32)
    z1g = nc.dram_tensor("z1g", [NROWS, ROWE], F32, addr_space="Shared")
    hg = nc.dram_tensor("hg", [NROWS, ROWE], F32, addr_space="Shared")

    stsizes = [512] * 12 + [128]  # node supertiles (6272 total)

    with tile.TileContext(nc) as tc:
        with (
            tc.tile_pool(name="persist", bufs=1) as pp,
            tc.tile_pool(name="xload", bufs=3) as xp,
            tc.tile_pool(name="gather", bufs=9) as gp_pool,
            tc.tile_pool(name="work", bufs=2) as wp,
            tc.tile_pool(name="outsb", bufs=3) as op_pool,
            tc.tile_pool(name="l1ps", bufs=2, space="PSUM") as l1ps,
            tc.tile_pool(name="trps", bufs=2, space="PSUM") as trps,
            tc.tile_pool(name="outps", bufs=2, space="PSUM") as outps,
        ):
            # ---- persistent loads ----
            w1_sb = pp.tile([XCH, NCHUNK_IN * 64], BF16)
            nc.sync.dma_start(w1_sb[:], w1[:])
            w2_sb = pp.tile([64, IN_CH], F32R)
            nc.sync.dma_start(w2_sb[:], w2[:])
            idxs_sb = pp.tile([P, totc * 8], I16)
            nc.sync.dma_start(idxs_sb[:], idxs[:])
            dinv_sb = pp.tile([P, T], F32)
            nc.sync.dma_start(dinv_sb[:], dinv_in[:])
            ident = pp.tile([P, P], F32)
            make_identity(nc, ident)

            z1r_sb = pp.tile([P, T * 64], F32)  # [z1 | r1+b1] per tile
            slotred = pp.tile([P, T * HID], F32)
            slotred2 = pp.tile([P, T * HID], F32)
            h_sb = pp.tile([P, T * HID], F32)
            tmp_sb = pp.tile([P, T * HID], F32)
            ag2h = pp.tile([P, T * 64], F32)  # [agg2 | h] per tile

            w1v = w1_sb[:].rearrange("p (k c) -> p k c", k=NCHUNK_IN)

            # ---- layer-1 matmul: z1|r1b = xTe_aug @ W1cat ----
            tglob = 0
            for st, stn in enumerate(stsizes):
                xsb = xp.tile([XCH, NCHUNK_IN, 512], BF16, tag="xsb")
                src = xTe.ap().rearrange("(k q) n -> q k n", q=XCH)[
                    :, :, st * 512 : st * 512 + stn
                ]
                nc.sync.dma_start(xsb[:, :, :stn], src)
                for tloc in range(stn // 128):
                    ps = l1ps.tile([P, 64], F32, space="PSUM")
                    for k in range(NCHUNK_IN):
                        nc.tensor.matmul(
                            out=ps[:],
                            lhsT=xsb[:, k, tloc * 128 : (tloc + 1) * 128],
                            rhs=w1v[:, k, :],
                            start=(k == 0),
                            stop=(k == NCHUNK_IN - 1),
                        )
                    nc.vector.tensor_copy(
                        z1r_sb[:, tglob * 64 : (tglob + 1) * 64], ps[:]
                    )
                    tglob += 1

            # ---- store z1 rows, AllGather ----
            z1v = z1r_sb[:].rearrange("p (t d) -> p t d", t=T)
            z1dst = z1loc.ap().rearrange("(t p) c -> p t c", p=P)[:, :, 0:HID]
            nc.sync.dma_start(z1dst, z1v[:, :, 0:HID])
            nc.gpsimd.collective_compute(
                "AllGather",
                mybir.AluOpType.bypass,
                replica_groups=[list(range(NCORES))],
                ins=[z1loc.ap().opt()],
                outs=[z1g.ap().opt()],
            )

            # ---- gather + reduce helper ----
            def gather_layer(table, dest_red, sems, chunk_cb=None, order=None):
                if order is None:
                    order = list(range(len(chunk_meta)))
                gbufs = {}
                for ci in order:
                    icol, cols_ch, runs = chunk_meta[ci]
                    gbuf = gp_pool.tile([P, MAXCOLS, HID], F32, tag="gbuf")
                    gbufs[ci] = gbuf
                    with tc.tile_critical(no_gpsimd_drain=True):
                        dma_gather_raw(
                            nc,
                            gbuf[:, :cols_ch, :],
                            table[BASE:, :],
                            idxs_sb[:, icol * 8 : (icol + cols_ch) * 8],
                            num_idxs=cols_ch * 128,
                            elem_size=HID,
                            elem_step=ROWE,
                            queue_num=ci % 4,
                        ).then_inc(sems[ci], 16)
                for ci in order:
                    icol, cols_ch, runs = chunk_meta[ci]
                    gbuf = gbufs[ci]
                    with tc.tile_critical():
                        nc.vector.wait_ge(sems[ci], 16)
                        for (coff, nt, kk, t0) in runs:
                            inv = gbuf[:, coff : coff + nt * kk, :].rearrange(
                                "p (t k) c -> p t c k", k=kk
                            )
                            nc.vector.tensor_reduce(
                                out=dest_red[:, t0 * HID : (t0 + nt) * HID],
                                in_=inv,
                                axis=mybir.AxisListType.X,
                                op=mybir.AluOpType.add,
                            )
                    if chunk_cb is not None:
                        chunk_cb(ci)

            tmp_v = tmp_sb[:].rearrange("p (t c) -> p t c", t=T)
            h_v = h_sb[:].rearrange("p (t c) -> p t c", t=T)
            ag2h_v = ag2h[:].rearrange("p (t d) -> p t d", t=T)
            hdst_all = hloc.ap().rearrange("(t p) c -> p t c", p=P)

            def l1_chunk(ci):
                a, b = chunks[ci]
                nt = b - a
                sl = slice(a * HID, b * HID)
                tv = tmp_sb[:, sl].rearrange("p (t c) -> p t c", t=nt)
                db = dinv_sb[:, a:b].to_broadcast([P, nt, HID])
                nc.vector.tensor_scalar(
                    out=tv, in0=z1v[:, a:b, 0:HID], scalar1=2.0, scalar2=None,
                    op0=mybir.AluOpType.mult,
                )
                nc.vector.tensor_tensor(
                    out=tmp_sb[:, sl], in0=tmp_sb[:, sl], in1=slotred[:, sl],
                    op=mybir.AluOpType.add,
                )
                nc.vector.tensor_tensor(
                    out=tv, in0=tv, in1=db, op=mybir.AluOpType.mult
                )
                nc.vector.tensor_tensor(
                    out=tv, in0=tv, in1=z1v[:, a:b, HID:64],
                    op=mybir.AluOpType.add,
                )
                nc.vector.tensor_scalar(
                    out=h_sb[:, sl], in0=tmp_sb[:, sl], scalar1=0.0, scalar2=None,
                    op0=mybir.AluOpType.max,
                )
                nc.vector.tensor_copy(ag2h_v[:, a:b, HID:64], h_v[:, a:b, :])
                nc.sync.dma_start(hdst_all[:, a:b, 0:HID], h_v[:, a:b, :])

            rev = list(range(len(chunk_meta)))[::-1]
            sems1 = [nc.alloc_semaphore(f"g1_{i}") for i in range(len(chunk_meta))]
            gather_layer(z1g, slotred, sems1, chunk_cb=l1_chunk, order=rev)
            nc.gpsimd.collective_compute(
                "AllGather",
                mybir.AluOpType.bypass,
                replica_groups=[list(range(NCORES))],
                ins=[hloc.ap().opt()],
                outs=[hg.ap().opt()],
            )

            # ---- L2: per-chunk assembly + output pipeline ----
            def l2_chunk(ci):
                a, b = chunks[ci]
                nt = b - a
                sl = slice(a * HID, b * HID)
                tv = tmp_sb[:, sl].rearrange("p (t c) -> p t c", t=nt)
                nc.vector.tensor_scalar(
                    out=tmp_sb[:, sl], in0=h_sb[:, sl], scalar1=2.0, scalar2=None,
                    op0=mybir.AluOpType.mult,
                )
                nc.vector.tensor_tensor(
                    out=tmp_sb[:, sl], in0=tmp_sb[:, sl], in1=slotred2[:, sl],
                    op=mybir.AluOpType.add,
                )
                nc.vector.tensor_tensor(
                    out=ag2h_v[:, a:b, 0:HID], in0=tv,
                    in1=dinv_sb[:, a:b].to_broadcast([P, nt, HID]),
                    op=mybir.AluOpType.mult,
                )
                for t in range(a, b):
                    tp = trps.tile([64, P], F32, space="PSUM")
                    nc.tensor.transpose(
                        out=tp[:], in_=ag2h[:, t * 64 : (t + 1) * 64],
                        identity=ident[:],
                    )
                    catT = wp.tile([64, P], F32R, tag="catT")
                    nc.vector.tensor_copy(catT[:], tp[:])
                    pso = outps.tile([P, IN_CH], F32, space="PSUM")
                    nc.tensor.matmul(
                        out=pso[:, 0:512], lhsT=catT[:], rhs=w2_sb[:, 0:512],
                        start=True, stop=True,
                    )
                    nc.tensor.matmul(
                        out=pso[:, 512:IN_CH], lhsT=catT[:], rhs=w2_sb[:, 512:IN_CH],
                        start=True, stop=True,
                    )
                    osb = op_pool.tile([P, IN_CH], F32, tag="osb")
                    nc.vector.tensor_copy(osb[:], pso[:])
                    nc.sync.dma_start(out[t * P : (t + 1) * P, :], osb[:])

            sems2 = [nc.alloc_semaphore(f"g2_{i}") for i in range(len(chunk_meta))]
            gather_layer(hg, slotred2, sems2, chunk_cb=l2_chunk, order=rev)

    nc.compile()
    return nc


_CACHE = {}


def kernel(x, edge_index, W_out1, b_out1, W_root1, W_out2, b_out2, W_root2):
    x = np.asarray(x, dtype=np.float32)
    layout, per_core_idxs, dpc, xTe, g_of_node = _host_prep(x, edge_index)

    # weight tensors
    w1cat = np.zeros((XROWS, 64), dtype=np.float32)
    w1cat[:IN_CH, :HID] = np.asarray(W_out1, dtype=np.float32).T
    w1cat[:IN_CH, HID:] = np.asarray(W_root1, dtype=np.float32).T
    w1cat[IN_CH, HID:] = np.asarray(b_out1, dtype=np.float32)
    # rearrange to [XCH, NCHUNK_IN*64]: [p, k*64+c] = w1cat[k*XCH+p, c]
    import ml_dtypes

    w1_dev = np.ascontiguousarray(
        w1cat.reshape(NCHUNK_IN, XCH, 64).transpose(1, 0, 2).reshape(XCH, -1)
    ).astype(ml_dtypes.bfloat16)
    w2_dev = np.concatenate(
        [np.asarray(W_out2, dtype=np.float32).T, np.asarray(W_root2, dtype=np.float32).T],
        axis=0,
    )  # [64, 770]

    key = (tuple(int(k) for k in layout["K"]), tuple(layout["chunks"]))
    if key not in _CACHE:
        _CACHE[key] = build(layout)
    nc = _CACHE[key]

    in_maps = []
    for c in range(NCORES):
        in_maps.append(
            {
                "xTe": np.ascontiguousarray(xTe[:, c * LOCAL : (c + 1) * LOCAL]).astype(ml_dtypes.bfloat16),
                "w1": w1_dev,
                "w2": np.ascontiguousarray(w2_dev),
                "idxs": per_core_idxs[c],
                "dinv": dpc[c],
            }
        )

    trace = os.environ.get("GCN_TRACE", "0") == "1"
    res = run_bass_kernel_spmd(
        nc, in_maps, core_ids=list(range(NCORES)), trace=trace
    )
    if trace and res.exec_time_ns is not None:
        print(f"HW exec time: {res.exec_time_ns} ns")
        kernel.last_exec_time_ns = res.exec_time_ns

    out = np.empty((N, IN_CH), dtype=np.float32)
    # table row g -> node
    node_of_g = np.full(NROWS, -1, dtype=np.int64)
    node_of_g[g_of_node] = np.arange(N)
    for c in range(NCORES):
        gs = np.arange(c * LOCAL, (c + 1) * LOCAL)
        nodes = node_of_g[gs]
        m = nodes >= 0
        out[nodes[m]] = res.results[c]["out"][m]
    out += np.asarray(b_out2, dtype=np.float32)[None, :]
    return out
